# revision 62
# baseline (speedup 1.0000x reference)
"""NetVLAD-style vq_codebook kernel for 8 Trainium2 NeuronCores.

Reference computation (per full input):
  assn = BN(x @ clusters); softmax over 80 clusters, drop 16 ghosts
  vlad[b,d,k] = sum_n assn[b,n,k] x[b,n,d] - a_sum[b,k]*clusters2[d,k]
  intra-normalize over d, flatten, global L2 normalize -> (B, D*K)

Sharding: data-parallel over batch B (B/8 batches per core). BatchNorm
statistics (sum and sum-of-squares per cluster column) are all-reduced
across the 8 cores (2*80 floats). Everything else is local.

Key structure (v2, redesigned around the engine cost model):
 - x cast-loaded fp32->fp16 by SWDGE DMA in token-partition layout.
 - x^T (d-partition) via PE transposes (is_transpose matmuls writing
   fp16 PSUM), software-pipelined at 2-tile granularity and batch-
   evacuated to SBUF by DVE and Act, alternating per quarter-group.
 - assignment matmul per token tile: 4 accumulating (128x128)@(128x80)
   fp16 matmuls. BN sum-of-squares via a long PE ones-matmul group;
   BN sums via DVE free-axis reduces + a PE partition reduce, sharing
   one PSUM bank with strictly sequential accumulation groups.
 - softmax: scale/shift as fp16 2x DVE tensor-tensor ops, Exp on Act
   (one activation table for ln/exp/copy/square -> a single load),
   pairwise-halved fp16 denominator, 1/sqrt as exp(-0.5*ln(x)).
 - vlad with x stationary in a d=4p+c column layout so the final DMA
   writes 1KB-contiguous runs; a_sum accumulated directly as [1,64]
   before the vlad groups so a_sum*clusters2 overlaps them.
 - global L2 norm folded analytically: after intra-normalization the
   flat norm is exactly sqrt(64), so y = v * rsqrt(64*nrm2[k]).
 - one serial neck (stats hop + BN math) between the assignment pass
   and the softmax/vlad pass; batch-0 softmax runs in small chunks so
   the first vlad matmuls start early.
"""

import sys

for _p in ("/opt/trn_rl_repo", "/root/.axon_site/_ro/trn_rl_repo"):
    if _p not in sys.path:
        sys.path.insert(0, _p)

import numpy as np

import concourse.bacc as bacc
import concourse.mybir as mybir
import concourse.tile as tile
from concourse.bass_utils import run_bass_kernel_spmd

F32 = mybir.dt.float32
F16 = mybir.dt.float16
AX = mybir.AxisListType
OP = mybir.AluOpType
ACTF = mybir.ActivationFunctionType

N_CORES = 8
D = 512
KG = 80          # clusters + ghosts
K = 64           # real clusters
N_SEQ = 2048
TPB = N_SEQ // 128   # token tiles per batch = 16
BN_EPS = 1e-5

# Tunables
import os as _os
XBAR_QUARTERS = tuple(
    int(v) for v in _os.environ.get("K_XBAR", "").split(",") if v)
LAG = int(_os.environ.get("K_LAG", "5"))
LOADS = tuple(int(v) for v in _os.environ.get("K_LOADS", "4,4").split(","))
PXT_BUFS = int(_os.environ.get("K_PXT", "4"))


def build(b_loc=4, n_cores=N_CORES, with_collective=True):
    """Build the per-core program. b_loc = batches per core."""
    nt = b_loc * TPB                # token tiles per core
    tok = nt * 128                  # tokens per core
    total_tok = tok * n_cores       # global token count for BN stats
    NH = nt // 4                    # half-groups (4 tiles each)

    nc = bacc.Bacc("TRN2", target_bir_lowering=False, debug=False,
                   dynamic_dma_scratch_size=65536)

    x = nc.declare_dram_parameter("x", [tok, D], F32, isOutput=False)
    cl = nc.declare_dram_parameter("clusters", [D, KG], F32, isOutput=False)
    c2 = nc.declare_dram_parameter("clusters2", [D, K], F32, isOutput=False)
    gam = nc.declare_dram_parameter("bn_gamma", [1, KG], F32, isOutput=False)
    bet = nc.declare_dram_parameter("bn_beta", [1, KG], F32, isOutput=False)
    y = nc.declare_dram_parameter("y", [b_loc, D * K], F32, isOutput=True)

    eye_c = nc.inline_tensor(np.eye(128, dtype=np.float16), name="c_eye")

    with tile.TileContext(nc) as tc:
        with (
            tc.tile_pool(name="persist", bufs=1) as persist,
            tc.tile_pool(name="work", bufs=4) as work,
            tc.tile_pool(name="dram", bufs=1, space="DRAM") as dram,
        ):
            # ---- persistent SBUF tensors ----
            xh = persist.tile([128, nt, D], F16, name="xh")
            assn = persist.tile([128, nt, KG], F16, name="assn")
            asq = persist.tile([128, nt, KG], F16, name="asq")
            sm = persist.tile([128, nt, K], F16, name="sm")
            idn = persist.tile([128, 128], F16, name="idn")
            clh = persist.tile([128, 4, KG], F16, name="clh")
            c2n = persist.tile([128, 4, K], F16, name="c2n")
            ones16 = persist.tile([128, 1], F16, name="ones16")
            ones_row = persist.tile([1, 128], F16, name="ones_row")
            gamma = persist.tile([1, KG], F32, name="gamma")
            beta = persist.tile([1, KG], F32, name="beta")
            ss16 = persist.tile([1, 2 * KG], F16, name="ss16")
            bcB = persist.tile([128, 2 * KG], F16, name="bcB")
            stats_sb = persist.tile([1, 2 * KG], F32, name="stats_sb")
            stats_g = persist.tile([1, 2 * KG], F32, name="stats_g")
            actwarm = persist.tile([1, 1], F32, name="actwarm")
            eps_sb = persist.tile([1, 1], F32, name="eps_sb")

            stats_in = dram.tile([1, 2 * KG], F32, name="stats_in")
            stats_out = dram.tile([1, 2 * KG], F32, name="stats_out")

            # ---- phase 0: constants + x load/cast ----
            nc.sync.dma_start(gamma[:], gam[:, :])
            nc.sync.dma_start(beta[:], bet[:, :])
            nc.sync.dma_start(idn[:], eye_c.ap()[:, :])
            nc.vector.memset(ones16[:], 1.0)
            nc.vector.memset(ones_row[:], 1.0)
            nc.vector.memset(eps_sb[:], BN_EPS)
            # Pre-load the one activation table covering every function this
            # kernel uses (ln/exp/copy/square), so the table-load inserter
            # doesn't alternate between ln-only and exp-only sets. Best
            # effort: fall back to automatic insertion if the set is absent.
            try:
                from concourse.hw_specs import get_activation_tables
                tabs = get_activation_tables(nc.m.arch)
                set_id = list(tabs).index("natural_log_exp_and_others")
                nc.scalar.add_instruction(mybir.InstLoadActFuncSet(
                    name=nc.get_next_instruction_name(),
                    engine=mybir.EngineType.Activation,
                    act_func_set_id=set_id, ins=[], outs=[]))
            except (ImportError, ValueError, KeyError):
                pass
            # Touch the activation engine early so any residual table load
            # happens off the critical path.
            nc.scalar.activation(actwarm[:], gamma[:, :1], ACTF.Ln)

            # x cast-DMA (SWDGE casts fp32->fp16 in the DMA engines; HBM
            # read is the real cost). Small first chunks start the PE
            # transpose pipeline sooner.
            xr = x.ap().rearrange("(t p) d -> p t d", p=128)
            t0 = 0
            for sz in LOADS + (8,) * ((nt - sum(LOADS)) // 8):
                nc.gpsimd.dma_start(
                    xh[:, t0:t0 + sz, :], xr[:, t0:t0 + sz, :])
                t0 += sz
            assert t0 == nt
            # clusters via HWDGE (fp32) + DVE cast: the Pool/SWDGE queue is
            # saturated by the x loads, and clh is needed early.
            clf = work.tile([128, 4, KG], F32, name="clf", tag="clf", bufs=1)
            nc.sync.dma_start(
                clf[:], cl.ap().rearrange("(c p) k -> p c k", p=128))
            nc.vector.tensor_copy(clh[:], clf[:])
            # clusters2 in d=4p+c layout (matches vlad output partitioning);
            # not needed until the post stage, so SWDGE order is fine.
            nc.gpsimd.dma_start(
                c2n[:], c2.ap().rearrange("(p c) k -> p c k", c=4))

            # ---- phase A: transposes + assignment matmul + BN stats ----
            with tc.tile_pool(name="psA", bufs=2, space="PSUM") as psA:
                # separate banks so the token-sum group can run while the
                # sum-of-squares group is still accumulating (start=True
                # clears a whole bank's has_written bits)
                pstat_q = psA.tile([1, KG], F32, name="pstat_q",
                                   tag="st_q", bufs=1)
                pstat_s = psA.tile([1, KG], F32, name="pstat_s",
                                   tag="st_s", bufs=1)

                NQ = nt // 2            # quarter-groups (2 tiles each)
                xtbufs = {}
                p1bufs = {}

                def produce(q):
                    # xTsb for quarter q: [128, 8, 128] fp16 with block
                    # e = 4j + c holding x[tile 2q+j, 128c:128c+128]^T
                    xTsb = work.tile([128, 8, 128], F16, name=f"xT{q}",
                                     tag="xt", bufs=LAG + 2)
                    if q in XBAR_QUARTERS:
                        nc.sync.dma_start(xTsb[:, :, :],
                                          xh[:, 2 * q:2 * (q + 1), :],
                                          transpose=True)
                    else:
                        pxt = psA.tile([128, 8, 128], F16, name="pxt",
                                       tag="pxt", bufs=PXT_BUFS)
                        for j in range(2):
                            t = 2 * q + j
                            for c in range(4):
                                nc.tensor.transpose(
                                    pxt[:, 4 * j + c, :],
                                    xh[:, t, 128 * c:128 * (c + 1)], idn[:])
                        # batched PSUM->SBUF evacuation; alternate DVE/Act
                        if q % 2 == 0:
                            nc.vector.tensor_copy(xTsb[:], pxt[:])
                        else:
                            nc.scalar.activation(xTsb[:], pxt[:], ACTF.Copy)
                    xtbufs[q] = xTsb

                def consume(q):
                    xTsb = xtbufs.pop(q)
                    if q % 2 == 0:
                        p1bufs[q // 2] = psA.tile([128, 4, KG], F32,
                                                  name="p1", tag="p1", bufs=2)
                    p1 = p1bufs[q // 2]
                    for j in range(2):
                        for c in range(4):
                            nc.tensor.matmul(
                                p1[:, 2 * (q % 2) + j, :],
                                xTsb[:, 4 * j + c, :],
                                clh[:, c, :], start=(c == 0), stop=(c == 3),
                                skip_group_check=True)
                    if q % 2 == 1:
                        h = q // 2
                        sl = slice(4 * h, 4 * (h + 1))
                        nc.scalar.activation(assn[:, sl, :], p1[:], ACTF.Copy)
                        if h == NH - 1:
                            # Act square: DVE is backlogged at phase-A end
                            with nc.allow_low_precision("fp16 stats sq"):
                                nc.scalar.activation(asq[:, sl, :],
                                                     assn[:, sl, :],
                                                     ACTF.Square)
                        else:
                            nc.vector.tensor_tensor(asq[:, sl, :],
                                                    assn[:, sl, :],
                                                    assn[:, sl, :],
                                                    op=OP.mult)

                def stats(h):
                    for j in range(4):
                        t = 4 * h + j
                        nc.tensor.matmul(pstat_q[:], ones16[:],
                                         asq[:, t, :],
                                         start=(t == 0), stop=(t == nt - 1),
                                         skip_group_check=True)
                        if t >= 3 * nt // 4:
                            nc.tensor.matmul(pstat_s[:], ones16[:],
                                             assn[:, t, :],
                                             start=(t == 3 * nt // 4),
                                             stop=False,
                                             skip_group_check=True)

                sacc = persist.tile([128, KG], F16, name="sacc")

                def ssum(c):
                    # DVE free-axis partial sum of assn over 16 tiles
                    with nc.allow_low_precision("fp16 stats partials"):
                        if c == 0:
                            nc.vector.tensor_reduce(
                                sacc[:],
                                assn[:, :16, :].rearrange("p t k -> p k t"),
                                axis=AX.X, op=OP.add)
                            return
                        red = work.tile([128, KG], F16, name="red", tag="red",
                                        bufs=2)
                        nc.vector.tensor_reduce(
                            red[:],
                            assn[:, 16 * c:16 * (c + 1), :]
                            .rearrange("p t k -> p k t"),
                            axis=AX.X, op=OP.add)
                        nc.vector.tensor_tensor(sacc[:], sacc[:], red[:],
                                                op=OP.add)

                # Stats matmuls are emitted 3 half-groups behind the assn
                # evacuations they read: the PE queue is in-order, so a stats
                # matmul whose Act/DVE evacuation hasn't retired yet would
                # stall the whole PE pipeline.
                stats_done = 0
                ssum_done = 0
                for q in range(NQ + LAG):
                    if q < NQ:
                        produce(q)
                    if q >= LAG:
                        cq = q - LAG
                        consume(cq)
                        ready_h = (cq + 1) // 2 - 3
                        while stats_done < ready_h:
                            stats(stats_done)
                            stats_done += 1
                        while ssum_done < min(3, ready_h // 4):
                            ssum(ssum_done)
                            ssum_done += 1
                while stats_done < NH:
                    stats(stats_done)
                    stats_done += 1
                while ssum_done < 3:
                    ssum(ssum_done)
                    ssum_done += 1
                # close the token-sum group with the DVE partial (tiles 0-47)
                nc.tensor.matmul(pstat_s[:], ones16[:], sacc[:],
                                 start=False, stop=True,
                                 skip_group_check=True)

                # ---- neck: stats all-reduce + BN parameters ----
                # stats_sb layout: [sum_sq (q), sum (s)]
                nc.vector.tensor_copy(stats_sb[:, :KG], pstat_q[:])
                nc.vector.tensor_copy(stats_sb[:, KG:], pstat_s[:])

            if with_collective:
                nc.sync.dma_start(stats_in[:], stats_sb[:])
                nc.gpsimd.collective_compute(
                    "AllReduce", OP.add,
                    replica_groups=[list(range(n_cores))],
                    ins=[stats_in.opt()], outs=[stats_out.opt()])
                nc.sync.dma_start(stats_g[:], stats_out[:])
            else:
                # single-core stand-in for the collective hop
                nc.sync.dma_start(stats_g[:], stats_sb[:])

            t_s2 = work.tile([1, KG], F32, name="t_s2", tag="sv2", bufs=4)
            t_vr = work.tile([1, KG], F32, name="t_vr", tag="sv2", bufs=4)
            t_ln = work.tile([1, KG], F32, name="t_ln", tag="sv2", bufs=4)
            t_sc = work.tile([1, KG], F32, name="t_sc", tag="sv2", bufs=4)
            t_mc = work.tile([1, KG], F32, name="t_mc", tag="sv2", bufs=4)
            inv_n = 1.0 / float(total_tok)
            # var = inv_n*(q - inv_n*s^2); rsqrt via exp(-0.5 ln(.)) with the
            # inv_n factor folded into the Ln's scale operand
            q_row, s_row = stats_g[:, :KG], stats_g[:, KG:]
            nc.vector.tensor_tensor(t_s2[:], s_row, s_row, op=OP.mult)
            nc.vector.scalar_tensor_tensor(t_vr[:], t_s2[:], -inv_n, q_row,
                                           op0=OP.mult, op1=OP.add)
            nc.scalar.activation(t_ln[:], t_vr[:], ACTF.Ln, bias=eps_sb[:],
                                 scale=inv_n)
            nc.scalar.activation(t_sc[:], t_ln[:], ACTF.Exp, scale=-0.5)
            with nc.allow_low_precision("fp16 bn scale"):
                nc.vector.tensor_tensor(ss16[:, :KG], t_sc[:], gamma[:],
                                        op=OP.mult)
            # shift = beta - (inv_n*s)*scale_f32*gamma; use fp16 scale copy
            with nc.allow_low_precision("fp16 bn shift"):
                nc.vector.scalar_tensor_tensor(t_mc[:], s_row, inv_n,
                                               ss16[:, :KG],
                                               op0=OP.mult, op1=OP.mult)
                nc.vector.tensor_tensor(ss16[:, KG:], beta[:], t_mc[:],
                                        op=OP.subtract)
            nc.gpsimd.partition_broadcast(bcB[:], ss16[:])
            scale_b = bcB[:, :KG].rearrange("p (a k) -> p a k", a=1)
            shift_b = bcB[:, KG:].rearrange("p (a k) -> p a k", a=1)

            # ---- phase BC: softmax + vlad + normalize, per batch ----
            with (
                tc.tile_pool(name="psB", bufs=2, space="PSUM") as psB,
                tc.tile_pool(name="elem", bufs=2) as elem,
                tc.tile_pool(name="vpost", bufs=2) as vpost,
            ):
                state = {}
                tebufs = {}

                def te_chunk(t0, n, pool_add=False):
                    # te = exp(scale*assn + shift) for token tiles [t0,t0+n)
                    te = elem.tile([128, n, KG], F16, name="te",
                                   tag=f"te{t0}_{n}", bufs=1)
                    nc.vector.tensor_tensor(
                        te[:], assn[:, t0:t0 + n, :],
                        scale_b.to_broadcast([128, n, KG]), op=OP.mult)
                    eng = nc.gpsimd if pool_add else nc.vector
                    eng.tensor_tensor(
                        te[:], te[:], shift_b.to_broadcast([128, n, KG]),
                        op=OP.add)
                    nc.scalar.activation(te[:], te[:], ACTF.Exp)
                    tebufs[t0] = te

                def sm_chunk(t0, n):
                    # normalize: sm = te / sum_k te, dropping ghosts
                    te = tebufs.pop(t0)
                    # pairwise-add tree at fp16 2x before the 1x reduce
                    dh = work.tile([128, n, KG // 2], F16, name="dh",
                                   tag=f"dh{n}", bufs=3)
                    dh2 = work.tile([128, n, KG // 4], F16, name="dh2",
                                    tag=f"dh2{n}", bufs=3)
                    with nc.allow_low_precision("fp16 softmax denom"):
                        nc.vector.tensor_tensor(dh[:], te[:, :, :KG // 2],
                                                te[:, :, KG // 2:], op=OP.add)
                        nc.vector.tensor_tensor(dh2[:], dh[:, :, :KG // 4],
                                                dh[:, :, KG // 4:], op=OP.add)
                    denom = work.tile([128, n], F16, name="denom", tag=f"dn{n}",
                                      bufs=3)
                    with nc.allow_low_precision("fp16 softmax denom"):
                        nc.vector.tensor_reduce(denom[:], dh2[:], axis=AX.X,
                                                op=OP.add)
                    recip = work.tile([128, n], F16, name="recip", tag=f"rc{n}",
                                      bufs=3)
                    with nc.allow_low_precision("fp16 softmax recip"):
                        nc.vector.reciprocal(recip[:], denom[:])
                    nc.vector.tensor_tensor(
                        sm[:, t0:t0 + n, :], te[:, :, :K],
                        recip[:].rearrange("p (t a) -> p t a", a=1)
                        .to_broadcast([128, n, K]), op=OP.mult)

                # chunking: small first chunks so the first vlad matmuls can
                # start early; full batches later for low op overhead
                CHUNKS = [(0, 1), (1, 1), (2, 2), (4, 4), (8, 8)] + [
                    (b * TPB, TPB) for b in range(1, b_loc - 1)] + [
                    ((b_loc - 1) * TPB, TPB // 2),
                    ((b_loc - 1) * TPB + TPB // 2, TPB // 2)]

                def mm_stage(b):
                    t0 = b * TPB
                    pv = psB.tile([128, 4, K], F32, name="pv", tag="pv")
                    pas = psB.tile([1, K], F32, name="pas", tag="pas")
                    # a_sum first: its PSUM lands while the vlad c-groups
                    # stream, so av is ready before the last c-group stops
                    for i in range(TPB):
                        nc.tensor.matmul(pas[:], ones16[:], sm[:, t0 + i, :],
                                         start=(i == 0), stop=(i == TPB - 1),
                                         skip_group_check=True)
                    pa16 = work.tile([1, K], F16, name="pa16", tag="pa16",
                                     bufs=2)
                    with nc.allow_low_precision("fp16 a_sum"):
                        nc.scalar.activation(pa16[:], pas[:], ACTF.Copy)
                    av = vpost.tile([128, 4, K], F16, name="av", tag="av")
                    if b == b_loc - 1:
                        # last batch: broadcast via PE + DVE to skip the Pool
                        # round-trips on the tail-critical path
                        pamP = psB.tile([128, K], F32, name="pamP", tag="pamP")
                        nc.tensor.matmul(pamP[:], ones_row[:], pa16[:],
                                         start=True, stop=True,
                                         skip_group_check=True)
                        nc.vector.tensor_tensor(
                            av[:], c2n[:],
                            pamP[:].rearrange("p (a k) -> p a k", a=1)
                            .to_broadcast([128, 4, K]), op=OP.mult)
                    else:
                        pamB = vpost.tile([128, K], F16, name="pamB",
                                          tag="pam")
                        nc.gpsimd.partition_broadcast(pamB[:], pa16[:])
                        nc.gpsimd.tensor_tensor(
                            av[:], c2n[:],
                            pamB[:].rearrange("p (a k) -> p a k", a=1)
                            .to_broadcast([128, 4, K]), op=OP.mult)
                    # vlad: x stationary with d = 4p + c column layout
                    for c in range(4):
                        for i in range(TPB):
                            t = t0 + i
                            nc.tensor.matmul(
                                pv[:, c, :],
                                xh[:, t, c::4],
                                sm[:, t, :],
                                start=(i == 0), stop=(i == TPB - 1),
                                skip_group_check=True)
                    state[b] = (pv, av)

                def post_stage(b):
                    pv, av = state.pop(b)
                    v = vpost.tile([128, 4, K], F16, name="v", tag="v")
                    sq = vpost.tile([128, 4, K], F16, name="sq", tag="sq")
                    pnrm = psB.tile([1, K], F32, name="pnrm", tag="pnrm")
                    # halves over the c dim: v/sq/pnrm for c<2 overlap the
                    # c2/c3 vlad matmuls of this batch
                    for hc in range(2):
                        cs = slice(2 * hc, 2 * hc + 2)
                        with nc.allow_low_precision("fp16 vlad residual"):
                            nc.vector.tensor_tensor(v[:, cs, :], pv[:, cs, :],
                                                    av[:, cs, :],
                                                    op=OP.subtract)
                        with nc.allow_low_precision("fp16 norm squares"):
                            nc.scalar.activation(sq[:, cs, :], v[:, cs, :],
                                                 ACTF.Square)
                        for c in range(2 * hc, 2 * hc + 2):
                            nc.tensor.matmul(pnrm[:], ones16[:], sq[:, c, :],
                                             start=(c == 0), stop=(c == 3),
                                             skip_group_check=True)
                    # y = v * rsqrt(64*nrm2): intra-norm and global L2 norm
                    # folded (flat norm is exactly sqrt(64) post intra-norm)
                    rnl = work.tile([1, K], F32, name="rnl", tag="rnl")
                    nc.scalar.activation(rnl[:], pnrm[:], ACTF.Ln, scale=64.0)
                    rn16 = work.tile([1, K], F16, name="rn16", tag="rn16")
                    with nc.allow_low_precision("fp16 norm scale"):
                        nc.scalar.activation(rn16[:], rnl[:], ACTF.Exp,
                                             scale=-0.5)
                    vf = vpost.tile([128, 4, K], F32, name="vf", tag="vf")
                    yb = y[b, :].rearrange("(p c k) -> p c k", p=128, k=K)
                    if b == b_loc - 1:
                        # last batch: PE broadcast + DVE scale + split y
                        # write to shorten the tail-critical chain
                        prnP = psB.tile([128, K], F32, name="prnP", tag="pamP")
                        nc.tensor.matmul(prnP[:], ones_row[:], rn16[:],
                                         start=True, stop=True,
                                         skip_group_check=True)
                        prnPv = prnP[:].rearrange("p (a k) -> p a k", a=1)
                        for hc in range(2):
                            cs = slice(2 * hc, 2 * hc + 2)
                            nc.vector.tensor_tensor(
                                vf[:, cs, :], v[:, cs, :],
                                prnPv.to_broadcast([128, 2, K]), op=OP.mult)
                            nc.sync.dma_start(yb[:, cs, :], vf[:, cs, :])
                    else:
                        prnB = vpost.tile([128, K], F16, name="prnB",
                                          tag="prn")
                        nc.gpsimd.partition_broadcast(prnB[:], rn16[:])
                        prnBv = prnB[:].rearrange("p (a k) -> p a k", a=1)
                        nc.gpsimd.tensor_tensor(
                            vf[:], v[:], prnBv.to_broadcast([128, 4, K]),
                            op=OP.mult)
                        nc.sync.dma_start(yb[:, :, :], vf[:])

                # Skew-by-one software pipeline: each sm chunk is emitted one
                # te-chunk later so the DVE stream never waits on an Act exp;
                # vlad (PE) and post stages weave in as batches complete.
                nch = len(CHUNKS)
                done_b = 0
                for i in range(nch + 1):
                    if i < nch:
                        t0, n = CHUNKS[i]
                        te_chunk(t0, n, pool_add=False)
                    if i >= 1:
                        t0, n = CHUNKS[i - 1]
                        sm_chunk(t0, n)
                        if (t0 + n) % TPB == 0:     # batch done_b fully sm'd
                            if done_b >= 1:
                                post_stage(done_b - 1)
                            mm_stage(done_b)
                            done_b += 1
                post_stage(b_loc - 1)
    nc.compile()
    return nc


_CACHE = {}


def _get(b_loc, n_cores, with_collective):
    key = (b_loc, n_cores, with_collective)
    if key not in _CACHE:
        _CACHE[key] = build(b_loc, n_cores, with_collective)
    return _CACHE[key]


def make_in_maps(x, clusters, clusters2, bn_gamma, bn_beta, n_cores=N_CORES):
    B = x.shape[0]
    b_loc = B // n_cores
    shared = {
        "clusters": np.ascontiguousarray(clusters, np.float32),
        "clusters2": np.ascontiguousarray(
            np.asarray(clusters2).reshape(D, K), np.float32),
        "bn_gamma": np.ascontiguousarray(
            np.asarray(bn_gamma).reshape(1, KG), np.float32),
        "bn_beta": np.ascontiguousarray(
            np.asarray(bn_beta).reshape(1, KG), np.float32),
    }
    in_maps = []
    for i in range(n_cores):
        m = dict(shared)
        m["x"] = np.ascontiguousarray(
            np.asarray(x[i * b_loc:(i + 1) * b_loc]).reshape(
                b_loc * N_SEQ, D), np.float32)
        in_maps.append(m)
    return in_maps


def kernel(x, clusters, clusters2, bn_gamma, bn_beta):
    B, N, Dd = x.shape
    assert (N, Dd) == (N_SEQ, D) and B % N_CORES == 0
    b_loc = B // N_CORES
    nc = _get(b_loc, N_CORES, True)
    in_maps = make_in_maps(x, clusters, clusters2, bn_gamma, bn_beta)
    res = run_bass_kernel_spmd(nc, in_maps, core_ids=list(range(N_CORES)))
    out = np.concatenate([res.results[i]["y"] for i in range(N_CORES)], axis=0)
    return out


# revision 68
# speedup vs baseline: 1.0236x; 1.0236x over previous
"""NetVLAD-style vq_codebook kernel for 8 Trainium2 NeuronCores.

Reference computation (per full input):
  assn = BN(x @ clusters); softmax over 80 clusters, drop 16 ghosts
  vlad[b,d,k] = sum_n assn[b,n,k] x[b,n,d] - a_sum[b,k]*clusters2[d,k]
  intra-normalize over d, flatten, global L2 normalize -> (B, D*K)

Sharding: data-parallel over batch B (B/8 batches per core). BatchNorm
statistics (sum and sum-of-squares per cluster column) are all-reduced
across the 8 cores (2*80 floats). Everything else is local.

Key structure (v2, redesigned around the engine cost model):
 - x cast-loaded fp32->fp16 by SWDGE DMA in token-partition layout.
 - x^T (d-partition) via PE transposes (is_transpose matmuls writing
   fp16 PSUM), software-pipelined at 2-tile granularity and batch-
   evacuated to SBUF by DVE and Act, alternating per quarter-group.
 - assignment matmul per token tile: 4 accumulating (128x128)@(128x80)
   fp16 matmuls. BN sum-of-squares via a long PE ones-matmul group;
   BN sums via DVE free-axis reduces + a PE partition reduce, sharing
   one PSUM bank with strictly sequential accumulation groups.
 - softmax: scale/shift as fp16 2x DVE tensor-tensor ops, Exp on Act
   (one activation table for ln/exp/copy/square -> a single load),
   pairwise-halved fp16 denominator, 1/sqrt as exp(-0.5*ln(x)).
 - vlad with x stationary in a d=4p+c column layout so the final DMA
   writes 1KB-contiguous runs; a_sum accumulated directly as [1,64]
   before the vlad groups so a_sum*clusters2 overlaps them.
 - global L2 norm folded analytically: after intra-normalization the
   flat norm is exactly sqrt(64), so y = v * rsqrt(64*nrm2[k]).
 - one serial neck (stats hop + BN math) between the assignment pass
   and the softmax/vlad pass; batch-0 softmax runs in small chunks so
   the first vlad matmuls start early.
"""

import sys

for _p in ("/opt/trn_rl_repo", "/root/.axon_site/_ro/trn_rl_repo"):
    if _p not in sys.path:
        sys.path.insert(0, _p)

import numpy as np

import concourse.bacc as bacc
import concourse.mybir as mybir
import concourse.tile as tile
from concourse.bass_utils import run_bass_kernel_spmd

F32 = mybir.dt.float32
F16 = mybir.dt.float16
AX = mybir.AxisListType
OP = mybir.AluOpType
ACTF = mybir.ActivationFunctionType

N_CORES = 8
D = 512
KG = 80          # clusters + ghosts
K = 64           # real clusters
N_SEQ = 2048
TPB = N_SEQ // 128   # token tiles per batch = 16
BN_EPS = 1e-5

# Tunables
import os as _os
XBAR_QUARTERS = tuple(
    int(v) for v in _os.environ.get("K_XBAR", "").split(",") if v)
LAG = int(_os.environ.get("K_LAG", "5"))
LOADS = tuple(int(v) for v in _os.environ.get("K_LOADS", "4,4").split(","))
PXT_BUFS = int(_os.environ.get("K_PXT", "4"))


def build(b_loc=4, n_cores=N_CORES, with_collective=True):
    """Build the per-core program. b_loc = batches per core."""
    nt = b_loc * TPB                # token tiles per core
    tok = nt * 128                  # tokens per core
    total_tok = tok * n_cores       # global token count for BN stats
    NH = nt // 4                    # half-groups (4 tiles each)

    nc = bacc.Bacc("TRN2", target_bir_lowering=False, debug=False,
                   dynamic_dma_scratch_size=65536)

    x = nc.declare_dram_parameter("x", [tok, D], F32, isOutput=False)
    cl = nc.declare_dram_parameter("clusters", [D, KG], F32, isOutput=False)
    c2 = nc.declare_dram_parameter("clusters2", [D, K], F32, isOutput=False)
    gam = nc.declare_dram_parameter("bn_gamma", [1, KG], F32, isOutput=False)
    bet = nc.declare_dram_parameter("bn_beta", [1, KG], F32, isOutput=False)
    y = nc.declare_dram_parameter("y", [b_loc, D * K], F32, isOutput=True)

    eye_c = nc.inline_tensor(np.eye(128, dtype=np.float16), name="c_eye")

    with tile.TileContext(nc) as tc:
        with (
            tc.tile_pool(name="persist", bufs=1) as persist,
            tc.tile_pool(name="work", bufs=4) as work,
            tc.tile_pool(name="dram", bufs=1, space="DRAM") as dram,
        ):
            # ---- persistent SBUF tensors ----
            xh = persist.tile([128, nt, D], F16, name="xh")
            # persistent x^T only for tiles 16-47 (quarters 8-23): the BC
            # re-matmul for batches 1-2 re-reads exactly these
            xTall = persist.tile([128, 2 * nt, 128], F16,
                                 name="xTall")
            clp = persist.tile([128, 4, KG], F16, name="clp")
            assn = persist.tile([128, nt, KG], F16, name="assn")
            asq = persist.tile([128, 16, KG], F16, name="asq")
            # rolling two-batch window: vlad(b) trails sm(b) by <1 batch
            sm = persist.tile([128, 2 * TPB, K], F16, name="sm")
            idn = persist.tile([128, 128], F16, name="idn")
            clh = persist.tile([128, 4, KG], F16, name="clh")
            c2n = persist.tile([128, 4, K], F16, name="c2n")
            ones16 = persist.tile([128, 1], F16, name="ones16")
            ones_row = persist.tile([1, 128], F16, name="ones_row")
            gamma = persist.tile([1, KG], F32, name="gamma")
            beta = persist.tile([1, KG], F32, name="beta")
            ss16 = persist.tile([1, 2 * KG], F16, name="ss16")
            bcB = persist.tile([128, 2 * KG], F16, name="bcB")
            stats_sb = persist.tile([1, 2 * KG], F32, name="stats_sb")
            stats_g = persist.tile([1, 2 * KG], F32, name="stats_g")
            actwarm = persist.tile([1, 1], F32, name="actwarm")
            eps_sb = persist.tile([1, 1], F32, name="eps_sb")

            stats_in = dram.tile([1, 2 * KG], F32, name="stats_in")
            stats_out = dram.tile([1, 2 * KG], F32, name="stats_out")

            # ---- phase 0: constants + x load/cast ----
            nc.sync.dma_start(gamma[:], gam[:, :])
            nc.sync.dma_start(beta[:], bet[:, :])
            nc.sync.dma_start(idn[:], eye_c.ap()[:, :])
            nc.vector.memset(ones16[:], 1.0)
            nc.vector.memset(ones_row[:], 1.0)
            nc.vector.memset(eps_sb[:], BN_EPS)
            # Pre-load the one activation table covering every function this
            # kernel uses (ln/exp/copy/square), so the table-load inserter
            # doesn't alternate between ln-only and exp-only sets. Best
            # effort: fall back to automatic insertion if the set is absent.
            try:
                from concourse.hw_specs import get_activation_tables
                tabs = get_activation_tables(nc.m.arch)
                set_id = list(tabs).index("natural_log_exp_and_others")
                nc.scalar.add_instruction(mybir.InstLoadActFuncSet(
                    name=nc.get_next_instruction_name(),
                    engine=mybir.EngineType.Activation,
                    act_func_set_id=set_id, ins=[], outs=[]))
            except (ImportError, ValueError, KeyError):
                pass
            # Touch the activation engine early so any residual table load
            # happens off the critical path.
            nc.scalar.activation(actwarm[:], gamma[:, :1], ACTF.Ln)

            # x cast-DMA (SWDGE casts fp32->fp16 in the DMA engines; HBM
            # read is the real cost). Small first chunks start the PE
            # transpose pipeline sooner.
            xr = x.ap().rearrange("(t p) d -> p t d", p=128)
            t0 = 0
            for sz in LOADS + (8,) * ((nt - sum(LOADS)) // 8):
                nc.gpsimd.dma_start(
                    xh[:, t0:t0 + sz, :], xr[:, t0:t0 + sz, :])
                t0 += sz
            assert t0 == nt
            # clusters via HWDGE (fp32) + DVE cast: the Pool/SWDGE queue is
            # saturated by the x loads, and clh is needed early.
            clf = work.tile([128, 4, KG], F32, name="clf", tag="clf", bufs=1)
            nc.sync.dma_start(
                clf[:], cl.ap().rearrange("(c p) k -> p c k", p=128))
            nc.vector.tensor_copy(clh[:], clf[:])
            # clusters2 in d=4p+c layout (matches vlad output partitioning);
            # not needed until the post stage, so SWDGE order is fine.
            nc.gpsimd.dma_start(
                c2n[:], c2.ap().rearrange("(p c) k -> p c k", c=4))

            # ---- phase A: transposes + assignment matmul + BN stats ----
            with tc.tile_pool(name="psA", bufs=2, space="PSUM") as psA:
                # separate banks so the token-sum group can run while the
                # sum-of-squares group is still accumulating (start=True
                # clears a whole bank's has_written bits)
                pstat_q = psA.tile([1, KG], F32, name="pstat_q",
                                   tag="st_q", bufs=1)
                pstat_s = psA.tile([1, KG], F32, name="pstat_s",
                                   tag="st_s", bufs=1)

                NQ = nt // 2            # quarter-groups (2 tiles each)
                xtbufs = {}
                p1bufs = {}

                def xt_dst(q):
                    # quarters 8-23 (tiles 16-47) persist in xTall for the BC
                    # re-matmul; the rest roll through small buffers
                    if 8 <= q < 24:
                        return xTall, slice(8 * (q - 8), 8 * (q - 8) + 8)
                    xt = work.tile([128, 8, 128], F16, name=f"xt{q}",
                                   tag="xt", bufs=LAG + 2)
                    return xt, slice(0, 8)

                def produce(q):
                    # block e = 4j + c holds x[tile 2q+j, 128c:128c+128]^T
                    dst, sl = xt_dst(q)
                    if q in XBAR_QUARTERS:
                        nc.sync.dma_start(dst[:, sl, :],
                                          xh[:, 2 * q:2 * (q + 1), :],
                                          transpose=True)
                    else:
                        pxt = psA.tile([128, 8, 128], F16, name="pxt",
                                       tag="pxt", bufs=PXT_BUFS)
                        for j in range(2):
                            t = 2 * q + j
                            for c in range(4):
                                nc.tensor.transpose(
                                    pxt[:, 4 * j + c, :],
                                    xh[:, t, 128 * c:128 * (c + 1)], idn[:])
                        # batched PSUM->SBUF evacuation; alternate DVE/Act
                        if q % 2 == 0:
                            nc.vector.tensor_copy(dst[:, sl, :], pxt[:])
                        else:
                            nc.scalar.activation(dst[:, sl, :], pxt[:],
                                                 ACTF.Copy)
                    xtbufs[q] = (dst, sl.start)

                def consume(q):
                    src_t, base = xtbufs.pop(q)
                    if q % 2 == 0:
                        p1bufs[q // 2] = psA.tile([128, 4, KG], F32,
                                                  name="p1", tag="p1", bufs=2)
                    p1 = p1bufs[q // 2]
                    for j in range(2):
                        for c in range(4):
                            nc.tensor.matmul(
                                p1[:, 2 * (q % 2) + j, :],
                                src_t[:, base + 4 * j + c, :],
                                clh[:, c, :], start=(c == 0), stop=(c == 3),
                                skip_group_check=True)
                    if q % 2 == 1:
                        h = q // 2
                        sl = slice(4 * h, 4 * (h + 1))
                        # asq is a rolling 4-half window (stats consume it
                        # within the 3-half emission lag)
                        sla = slice(4 * (h % 4), 4 * (h % 4) + 4)
                        nc.scalar.activation(assn[:, sl, :], p1[:], ACTF.Copy)
                        if h == NH - 1:
                            # Act square: DVE is backlogged at phase-A end
                            with nc.allow_low_precision("fp16 stats sq"):
                                nc.scalar.activation(asq[:, sla, :],
                                                     assn[:, sl, :],
                                                     ACTF.Square)
                        else:
                            nc.vector.tensor_tensor(asq[:, sla, :],
                                                    assn[:, sl, :],
                                                    assn[:, sl, :],
                                                    op=OP.mult)

                def stats(h):
                    for j in range(4):
                        t = 4 * h + j
                        nc.tensor.matmul(pstat_q[:], ones16[:],
                                         asq[:, t % 16, :],
                                         start=(t == 0), stop=(t == nt - 1),
                                         skip_group_check=True)
                        if t >= 3 * nt // 4:
                            nc.tensor.matmul(pstat_s[:], ones16[:],
                                             assn[:, t, :],
                                             start=(t == 3 * nt // 4),
                                             stop=False,
                                             skip_group_check=True)

                sacc = persist.tile([128, KG], F16, name="sacc")

                def ssum(c):
                    # DVE free-axis partial sum of assn over 16 tiles
                    with nc.allow_low_precision("fp16 stats partials"):
                        if c == 0:
                            nc.vector.tensor_reduce(
                                sacc[:],
                                assn[:, :16, :].rearrange("p t k -> p k t"),
                                axis=AX.X, op=OP.add)
                            return
                        red = work.tile([128, KG], F16, name="red", tag="red",
                                        bufs=2)
                        nc.vector.tensor_reduce(
                            red[:],
                            assn[:, 16 * c:16 * (c + 1), :]
                            .rearrange("p t k -> p k t"),
                            axis=AX.X, op=OP.add)
                        nc.vector.tensor_tensor(sacc[:], sacc[:], red[:],
                                                op=OP.add)

                # Stats matmuls are emitted 3 half-groups behind the assn
                # evacuations they read: the PE queue is in-order, so a stats
                # matmul whose Act/DVE evacuation hasn't retired yet would
                # stall the whole PE pipeline.
                stats_done = 0
                ssum_done = 0
                for q in range(NQ + LAG):
                    if q < NQ:
                        produce(q)
                    if q >= LAG:
                        cq = q - LAG
                        consume(cq)
                        ready_h = (cq + 1) // 2 - 3
                        while stats_done < ready_h:
                            stats(stats_done)
                            stats_done += 1
                        while ssum_done < min(3, ready_h // 4):
                            ssum(ssum_done)
                            ssum_done += 1
                while stats_done < NH:
                    stats(stats_done)
                    stats_done += 1
                while ssum_done < 3:
                    ssum(ssum_done)
                    ssum_done += 1
                # close the token-sum group with the DVE partial (tiles 0-47)
                nc.tensor.matmul(pstat_s[:], ones16[:], sacc[:],
                                 start=False, stop=True,
                                 skip_group_check=True)

                # ---- neck: stats all-reduce + BN parameters ----
                # stats_sb layout: [sum_sq (q), sum (s)]
                nc.vector.tensor_copy(stats_sb[:, :KG], pstat_q[:])
                nc.vector.tensor_copy(stats_sb[:, KG:], pstat_s[:])

            if with_collective:
                nc.sync.dma_start(stats_in[:], stats_sb[:])
                nc.gpsimd.collective_compute(
                    "AllReduce", OP.add,
                    replica_groups=[list(range(n_cores))],
                    ins=[stats_in.opt()], outs=[stats_out.opt()])
                nc.sync.dma_start(stats_g[:], stats_out[:])
            else:
                # single-core stand-in for the collective hop
                nc.sync.dma_start(stats_g[:], stats_sb[:])

            t_s2 = work.tile([1, KG], F32, name="t_s2", tag="sv2", bufs=4)
            t_vr = work.tile([1, KG], F32, name="t_vr", tag="sv2", bufs=4)
            t_ln = work.tile([1, KG], F32, name="t_ln", tag="sv2", bufs=4)
            t_sc = work.tile([1, KG], F32, name="t_sc", tag="sv2", bufs=4)
            t_mc = work.tile([1, KG], F32, name="t_mc", tag="sv2", bufs=4)
            inv_n = 1.0 / float(total_tok)
            # var = inv_n*(q - inv_n*s^2); rsqrt via exp(-0.5 ln(.)) with the
            # inv_n factor folded into the Ln's scale operand
            q_row, s_row = stats_g[:, :KG], stats_g[:, KG:]
            nc.vector.tensor_tensor(t_s2[:], s_row, s_row, op=OP.mult)
            nc.vector.scalar_tensor_tensor(t_vr[:], t_s2[:], -inv_n, q_row,
                                           op0=OP.mult, op1=OP.add)
            nc.scalar.activation(t_ln[:], t_vr[:], ACTF.Ln, bias=eps_sb[:],
                                 scale=inv_n)
            nc.scalar.activation(t_sc[:], t_ln[:], ACTF.Exp, scale=-0.5)
            with nc.allow_low_precision("fp16 bn scale"):
                nc.vector.tensor_tensor(ss16[:, :KG], t_sc[:], gamma[:],
                                        op=OP.mult)
            # shift = beta - (inv_n*s)*scale_f32*gamma; use fp16 scale copy
            with nc.allow_low_precision("fp16 bn shift"):
                nc.vector.scalar_tensor_tensor(t_mc[:], s_row, inv_n,
                                               ss16[:, :KG],
                                               op0=OP.mult, op1=OP.mult)
                nc.vector.tensor_tensor(ss16[:, KG:], beta[:], t_mc[:],
                                        op=OP.subtract)
            nc.gpsimd.partition_broadcast(bcB[:], ss16[:])
            scale_b = bcB[:, :KG].rearrange("p (a k) -> p a k", a=1)
            shift_b = bcB[:, KG:].rearrange("p (a k) -> p a k", a=1)
            # clusters pre-scaled by the BN scale: z = xT@clp + shift equals
            # scale*(x@cl) + shift, letting PE redo the scale/shift for some
            # batches and unload the DVE-bound softmax phase
            nc.vector.tensor_tensor(
                clp[:], clh[:],
                bcB[:, :KG].rearrange("p (a k) -> p a k", a=1)
                .to_broadcast([128, 4, KG]), op=OP.mult)

            # ---- phase BC: softmax + vlad + normalize, per batch ----
            with (
                tc.tile_pool(name="psB", bufs=2, space="PSUM") as psB,
                tc.tile_pool(name="elem", bufs=2) as elem,
                tc.tile_pool(name="vpost", bufs=2) as vpost,
            ):
                state = {}
                tebufs = {}

                def te_chunk(t0, n, pool_add=False):
                    # te = exp(scale*assn + shift) for token tiles [t0,t0+n)
                    te = elem.tile([128, n, KG], F16, name="te",
                                   tag=f"te{t0}_{n}", bufs=1)
                    if n == TPB and t0 // TPB in (1, 2):
                        # PE path: re-matmul with pre-scaled clusters into a
                        # shift-prefilled PSUM, Exp evacuates straight from
                        # PSUM. Rebalances work from the DVE-bound softmax
                        # onto the PE, which idles during this phase.
                        for g4 in range(n // 4):
                            zt = psB.tile([128, 4, KG], F32, name="zt",
                                          tag="zt", bufs=2)
                            for j in range(4):
                                t = t0 + 4 * g4 + j
                                qq, jj = divmod(t - TPB, 2)
                                nc.tensor.matmul(
                                    zt[:, j, :], ones_row[:], ss16[:, KG:],
                                    start=True, stop=False,
                                    skip_group_check=True)
                                for c in range(4):
                                    nc.tensor.matmul(
                                        zt[:, j, :],
                                        xTall[:, 8 * qq + 4 * jj + c, :],
                                        clp[:, c, :], start=False,
                                        stop=(c == 3),
                                        skip_group_check=True)
                            nc.scalar.activation(
                                te[:, 4 * g4:4 * (g4 + 1), :], zt[:],
                                ACTF.Exp)
                        tebufs[t0] = te
                        return
                    nc.vector.tensor_tensor(
                        te[:], assn[:, t0:t0 + n, :],
                        scale_b.to_broadcast([128, n, KG]), op=OP.mult)
                    eng = nc.gpsimd if pool_add else nc.vector
                    eng.tensor_tensor(
                        te[:], te[:], shift_b.to_broadcast([128, n, KG]),
                        op=OP.add)
                    nc.scalar.activation(te[:], te[:], ACTF.Exp)
                    tebufs[t0] = te

                def sm_chunk(t0, n):
                    # normalize: sm = te / sum_k te, dropping ghosts
                    te = tebufs.pop(t0)
                    # pairwise-add tree at fp16 2x before the 1x reduce
                    dh = work.tile([128, n, KG // 2], F16, name="dh",
                                   tag=f"dh{n}", bufs=2)
                    dh2 = work.tile([128, n, KG // 4], F16, name="dh2",
                                    tag=f"dh2{n}", bufs=2)
                    with nc.allow_low_precision("fp16 softmax denom"):
                        nc.vector.tensor_tensor(dh[:], te[:, :, :KG // 2],
                                                te[:, :, KG // 2:], op=OP.add)
                        nc.vector.tensor_tensor(dh2[:], dh[:, :, :KG // 4],
                                                dh[:, :, KG // 4:], op=OP.add)
                    denom = work.tile([128, n], F16, name="denom", tag=f"dn{n}",
                                      bufs=2)
                    with nc.allow_low_precision("fp16 softmax denom"):
                        nc.vector.tensor_reduce(denom[:], dh2[:], axis=AX.X,
                                                op=OP.add)
                    recip = work.tile([128, n], F16, name="recip", tag=f"rc{n}",
                                      bufs=2)
                    with nc.allow_low_precision("fp16 softmax recip"):
                        nc.vector.reciprocal(recip[:], denom[:])
                    s0 = t0 % (2 * TPB)
                    nc.vector.tensor_tensor(
                        sm[:, s0:s0 + n, :], te[:, :, :K],
                        recip[:].rearrange("p (t a) -> p t a", a=1)
                        .to_broadcast([128, n, K]), op=OP.mult)

                # chunking: small first chunks so the first vlad matmuls can
                # start early; full batches later for low op overhead
                CHUNKS = [(0, 1), (1, 1), (2, 2), (4, 4), (8, 8)] + [
                    (b * TPB, TPB) for b in range(1, b_loc - 1)] + [
                    ((b_loc - 1) * TPB, TPB // 2),
                    ((b_loc - 1) * TPB + TPB // 2, TPB // 2)]

                def mm_stage(b):
                    t0 = b * TPB
                    pv = psB.tile([128, 4, K], F32, name="pv", tag="pv")
                    pas = psB.tile([1, K], F32, name="pas", tag="pas", bufs=1)
                    # a_sum first: its PSUM lands while the vlad c-groups
                    # stream, so av is ready before the last c-group stops
                    for i in range(TPB):
                        nc.tensor.matmul(pas[:], ones16[:],
                                         sm[:, (t0 + i) % (2 * TPB), :],
                                         start=(i == 0), stop=(i == TPB - 1),
                                         skip_group_check=True)
                    pa16 = work.tile([1, K], F16, name="pa16", tag="pa16",
                                     bufs=2)
                    with nc.allow_low_precision("fp16 a_sum"):
                        nc.scalar.activation(pa16[:], pas[:], ACTF.Copy)
                    av = vpost.tile([128, 4, K], F16, name="av", tag="av")
                    if b == b_loc - 1:
                        # last batch: broadcast via PE + DVE to skip the Pool
                        # round-trips on the tail-critical path
                        pamP = psB.tile([128, K], F32, name="pamP",
                                        tag="pamP", bufs=1)
                        nc.tensor.matmul(pamP[:], ones_row[:], pa16[:],
                                         start=True, stop=True,
                                         skip_group_check=True)
                        nc.vector.tensor_tensor(
                            av[:], c2n[:],
                            pamP[:].rearrange("p (a k) -> p a k", a=1)
                            .to_broadcast([128, 4, K]), op=OP.mult)
                    else:
                        pamB = vpost.tile([128, K], F16, name="pamB",
                                          tag="pam")
                        nc.gpsimd.partition_broadcast(pamB[:], pa16[:])
                        nc.gpsimd.tensor_tensor(
                            av[:], c2n[:],
                            pamB[:].rearrange("p (a k) -> p a k", a=1)
                            .to_broadcast([128, 4, K]), op=OP.mult)
                    # vlad: x stationary with d = 4p + c column layout
                    for c in range(4):
                        for i in range(TPB):
                            t = t0 + i
                            nc.tensor.matmul(
                                pv[:, c, :],
                                xh[:, t, c::4],
                                sm[:, t % (2 * TPB), :],
                                start=(i == 0), stop=(i == TPB - 1),
                                skip_group_check=True)
                    state[b] = (pv, av)

                def post_stage(b):
                    pv, av = state.pop(b)
                    v = vpost.tile([128, 4, K], F16, name="v", tag="v")
                    sq = vpost.tile([128, 4, K], F16, name="sq", tag="sq")
                    pnrm = psB.tile([1, K], F32, name="pnrm", tag="pnrm",
                     bufs=1)
                    # halves over the c dim: v/sq/pnrm for c<2 overlap the
                    # c2/c3 vlad matmuls of this batch
                    for hc in range(2):
                        cs = slice(2 * hc, 2 * hc + 2)
                        with nc.allow_low_precision("fp16 vlad residual"):
                            nc.vector.tensor_tensor(v[:, cs, :], pv[:, cs, :],
                                                    av[:, cs, :],
                                                    op=OP.subtract)
                        with nc.allow_low_precision("fp16 norm squares"):
                            nc.scalar.activation(sq[:, cs, :], v[:, cs, :],
                                                 ACTF.Square)
                        for c in range(2 * hc, 2 * hc + 2):
                            nc.tensor.matmul(pnrm[:], ones16[:], sq[:, c, :],
                                             start=(c == 0), stop=(c == 3),
                                             skip_group_check=True)
                    # y = v * rsqrt(64*nrm2): intra-norm and global L2 norm
                    # folded (flat norm is exactly sqrt(64) post intra-norm)
                    rnl = work.tile([1, K], F32, name="rnl", tag="rnl")
                    nc.scalar.activation(rnl[:], pnrm[:], ACTF.Ln, scale=64.0)
                    rn16 = work.tile([1, K], F16, name="rn16", tag="rn16")
                    with nc.allow_low_precision("fp16 norm scale"):
                        nc.scalar.activation(rn16[:], rnl[:], ACTF.Exp,
                                             scale=-0.5)
                    vf = vpost.tile([128, 4, K], F32, name="vf", tag="vf")
                    yb = y[b, :].rearrange("(p c k) -> p c k", p=128, k=K)
                    if b == b_loc - 1:
                        # last batch: PE broadcast + DVE scale + split y
                        # write to shorten the tail-critical chain
                        prnP = psB.tile([128, K], F32, name="prnP",
                                        tag="pamP", bufs=1)
                        nc.tensor.matmul(prnP[:], ones_row[:], rn16[:],
                                         start=True, stop=True,
                                         skip_group_check=True)
                        prnPv = prnP[:].rearrange("p (a k) -> p a k", a=1)
                        for hc in range(2):
                            cs = slice(2 * hc, 2 * hc + 2)
                            nc.vector.tensor_tensor(
                                vf[:, cs, :], v[:, cs, :],
                                prnPv.to_broadcast([128, 2, K]), op=OP.mult)
                            nc.sync.dma_start(yb[:, cs, :], vf[:, cs, :])
                    else:
                        prnB = vpost.tile([128, K], F16, name="prnB",
                                          tag="prn")
                        nc.gpsimd.partition_broadcast(prnB[:], rn16[:])
                        prnBv = prnB[:].rearrange("p (a k) -> p a k", a=1)
                        nc.gpsimd.tensor_tensor(
                            vf[:], v[:], prnBv.to_broadcast([128, 4, K]),
                            op=OP.mult)
                        nc.sync.dma_start(yb[:, :, :], vf[:])

                # Skew-by-one software pipeline: each sm chunk is emitted one
                # te-chunk later so the DVE stream never waits on an Act exp;
                # vlad (PE) and post stages weave in as batches complete.
                nch = len(CHUNKS)
                done_b = 0
                for i in range(nch + 1):
                    if i < nch:
                        t0, n = CHUNKS[i]
                        te_chunk(t0, n, pool_add=False)
                    if i >= 1:
                        t0, n = CHUNKS[i - 1]
                        sm_chunk(t0, n)
                        if (t0 + n) % TPB == 0:     # batch done_b fully sm'd
                            if done_b >= 1:
                                post_stage(done_b - 1)
                            mm_stage(done_b)
                            done_b += 1
                post_stage(b_loc - 1)
    nc.compile()
    return nc


_CACHE = {}


def _get(b_loc, n_cores, with_collective):
    key = (b_loc, n_cores, with_collective)
    if key not in _CACHE:
        _CACHE[key] = build(b_loc, n_cores, with_collective)
    return _CACHE[key]


def make_in_maps(x, clusters, clusters2, bn_gamma, bn_beta, n_cores=N_CORES):
    B = x.shape[0]
    b_loc = B // n_cores
    shared = {
        "clusters": np.ascontiguousarray(clusters, np.float32),
        "clusters2": np.ascontiguousarray(
            np.asarray(clusters2).reshape(D, K), np.float32),
        "bn_gamma": np.ascontiguousarray(
            np.asarray(bn_gamma).reshape(1, KG), np.float32),
        "bn_beta": np.ascontiguousarray(
            np.asarray(bn_beta).reshape(1, KG), np.float32),
    }
    in_maps = []
    for i in range(n_cores):
        m = dict(shared)
        m["x"] = np.ascontiguousarray(
            np.asarray(x[i * b_loc:(i + 1) * b_loc]).reshape(
                b_loc * N_SEQ, D), np.float32)
        in_maps.append(m)
    return in_maps


def kernel(x, clusters, clusters2, bn_gamma, bn_beta):
    B, N, Dd = x.shape
    assert (N, Dd) == (N_SEQ, D) and B % N_CORES == 0
    b_loc = B // N_CORES
    nc = _get(b_loc, N_CORES, True)
    in_maps = make_in_maps(x, clusters, clusters2, bn_gamma, bn_beta)
    res = run_bass_kernel_spmd(nc, in_maps, core_ids=list(range(N_CORES)))
    out = np.concatenate([res.results[i]["y"] for i in range(N_CORES)], axis=0)
    return out


# revision 72
# speedup vs baseline: 1.0275x; 1.0038x over previous
"""NetVLAD-style vq_codebook kernel for 8 Trainium2 NeuronCores.

Reference computation (per full input):
  assn = BN(x @ clusters); softmax over 80 clusters, drop 16 ghosts
  vlad[b,d,k] = sum_n assn[b,n,k] x[b,n,d] - a_sum[b,k]*clusters2[d,k]
  intra-normalize over d, flatten, global L2 normalize -> (B, D*K)

Sharding: data-parallel over batch B (B/8 batches per core). BatchNorm
statistics (sum and sum-of-squares per cluster column) are all-reduced
across the 8 cores (2*80 floats). Everything else is local.

Key structure (v2, redesigned around the engine cost model):
 - x cast-loaded fp32->fp16 by SWDGE DMA in token-partition layout.
 - x^T (d-partition) via PE transposes (is_transpose matmuls writing
   fp16 PSUM), software-pipelined at 2-tile granularity and batch-
   evacuated to SBUF by DVE and Act, alternating per quarter-group.
 - assignment matmul per token tile: 4 accumulating (128x128)@(128x80)
   fp16 matmuls. BN sum-of-squares via a long PE ones-matmul group;
   BN sums via DVE free-axis reduces + a PE partition reduce, sharing
   one PSUM bank with strictly sequential accumulation groups.
 - softmax: scale/shift as fp16 2x DVE tensor-tensor ops, Exp on Act
   (one activation table for ln/exp/copy/square -> a single load),
   pairwise-halved fp16 denominator, 1/sqrt as exp(-0.5*ln(x)).
 - vlad with x stationary in a d=4p+c column layout so the final DMA
   writes 1KB-contiguous runs; a_sum accumulated directly as [1,64]
   before the vlad groups so a_sum*clusters2 overlaps them.
 - global L2 norm folded analytically: after intra-normalization the
   flat norm is exactly sqrt(64), so y = v * rsqrt(64*nrm2[k]).
 - one serial neck (stats hop + BN math) between the assignment pass
   and the softmax/vlad pass; batch-0 softmax runs in small chunks so
   the first vlad matmuls start early.
"""

import sys

for _p in ("/opt/trn_rl_repo", "/root/.axon_site/_ro/trn_rl_repo"):
    if _p not in sys.path:
        sys.path.insert(0, _p)

import numpy as np

import concourse.bacc as bacc
import concourse.mybir as mybir
import concourse.tile as tile
from concourse.bass_utils import run_bass_kernel_spmd

F32 = mybir.dt.float32
F16 = mybir.dt.float16
AX = mybir.AxisListType
OP = mybir.AluOpType
ACTF = mybir.ActivationFunctionType

N_CORES = 8
D = 512
KG = 80          # clusters + ghosts
K = 64           # real clusters
N_SEQ = 2048
TPB = N_SEQ // 128   # token tiles per batch = 16
BN_EPS = 1e-5

# Tunables
import os as _os
XBAR_QUARTERS = tuple(
    int(v) for v in _os.environ.get("K_XBAR", "").split(",") if v)
LAG = int(_os.environ.get("K_LAG", "5"))
LOADS = tuple(int(v) for v in _os.environ.get("K_LOADS", "4,4").split(","))
PXT_BUFS = int(_os.environ.get("K_PXT", "4"))


def build(b_loc=4, n_cores=N_CORES, with_collective=True):
    """Build the per-core program. b_loc = batches per core."""
    nt = b_loc * TPB                # token tiles per core
    tok = nt * 128                  # tokens per core
    total_tok = tok * n_cores       # global token count for BN stats
    NH = nt // 4                    # half-groups (4 tiles each)

    nc = bacc.Bacc("TRN2", target_bir_lowering=False, debug=False,
                   dynamic_dma_scratch_size=65536)

    x = nc.declare_dram_parameter("x", [tok, D], F32, isOutput=False)
    cl = nc.declare_dram_parameter("clusters", [D, KG], F32, isOutput=False)
    c2 = nc.declare_dram_parameter("clusters2", [D, K], F32, isOutput=False)
    gam = nc.declare_dram_parameter("bn_gamma", [1, KG], F32, isOutput=False)
    bet = nc.declare_dram_parameter("bn_beta", [1, KG], F32, isOutput=False)
    y = nc.declare_dram_parameter("y", [b_loc, D * K], F32, isOutput=True)

    eye_c = nc.inline_tensor(np.eye(128, dtype=np.float16), name="c_eye")

    with tile.TileContext(nc) as tc:
        with (
            tc.tile_pool(name="persist", bufs=1) as persist,
            tc.tile_pool(name="work", bufs=4) as work,
            tc.tile_pool(name="dram", bufs=1, space="DRAM") as dram,
        ):
            # ---- persistent SBUF tensors ----
            xh = persist.tile([128, nt, D], F16, name="xh")
            # persistent x^T only for tiles 32-63 (quarters 16-31): the BC
            # re-matmul for batches 2-3 re-reads exactly these
            xTall = persist.tile([128, 2 * nt, 128], F16,
                                 name="xTall")
            clp = persist.tile([128, 4, KG], F16, name="clp")
            assn = persist.tile([128, nt, KG], F16, name="assn")
            asq = persist.tile([128, 16, KG], F16, name="asq")
            # rolling two-batch window: vlad(b) trails sm(b) by <1 batch
            sm = persist.tile([128, 2 * TPB, K], F16, name="sm")
            idn = persist.tile([128, 128], F16, name="idn")
            clh = persist.tile([128, 4, KG], F16, name="clh")
            c2n = persist.tile([128, 4, K], F16, name="c2n")
            ones16 = persist.tile([128, 1], F16, name="ones16")
            ones_row = persist.tile([1, 128], F16, name="ones_row")
            gamma = persist.tile([1, KG], F32, name="gamma")
            beta = persist.tile([1, KG], F32, name="beta")
            ss16 = persist.tile([1, 2 * KG], F16, name="ss16")
            bcB = persist.tile([128, 2 * KG], F16, name="bcB")
            stats_sb = persist.tile([1, 2 * KG], F32, name="stats_sb")
            stats_g = persist.tile([1, 2 * KG], F32, name="stats_g")
            actwarm = persist.tile([1, 1], F32, name="actwarm")
            eps_sb = persist.tile([1, 1], F32, name="eps_sb")

            stats_in = dram.tile([1, 2 * KG], F32, name="stats_in")
            stats_out = dram.tile([1, 2 * KG], F32, name="stats_out")

            # ---- phase 0: constants + x load/cast ----
            nc.sync.dma_start(gamma[:], gam[:, :])
            nc.sync.dma_start(beta[:], bet[:, :])
            nc.sync.dma_start(idn[:], eye_c.ap()[:, :])
            nc.vector.memset(ones16[:], 1.0)
            nc.vector.memset(ones_row[:], 1.0)
            nc.vector.memset(eps_sb[:], BN_EPS)
            # Pre-load the one activation table covering every function this
            # kernel uses (ln/exp/copy/square), so the table-load inserter
            # doesn't alternate between ln-only and exp-only sets. Best
            # effort: fall back to automatic insertion if the set is absent.
            try:
                from concourse.hw_specs import get_activation_tables
                tabs = get_activation_tables(nc.m.arch)
                set_id = list(tabs).index("natural_log_exp_and_others")
                nc.scalar.add_instruction(mybir.InstLoadActFuncSet(
                    name=nc.get_next_instruction_name(),
                    engine=mybir.EngineType.Activation,
                    act_func_set_id=set_id, ins=[], outs=[]))
            except (ImportError, ValueError, KeyError):
                pass
            # Touch the activation engine early so any residual table load
            # happens off the critical path.
            nc.scalar.activation(actwarm[:], gamma[:, :1], ACTF.Ln)

            # x cast-DMA (SWDGE casts fp32->fp16 in the DMA engines; HBM
            # read is the real cost). Small first chunks start the PE
            # transpose pipeline sooner.
            xr = x.ap().rearrange("(t p) d -> p t d", p=128)
            t0 = 0
            for sz in LOADS + (8,) * ((nt - sum(LOADS)) // 8):
                nc.gpsimd.dma_start(
                    xh[:, t0:t0 + sz, :], xr[:, t0:t0 + sz, :])
                t0 += sz
            assert t0 == nt
            # clusters via HWDGE (fp32) + DVE cast: the Pool/SWDGE queue is
            # saturated by the x loads, and clh is needed early.
            clf = work.tile([128, 4, KG], F32, name="clf", tag="clf", bufs=1)
            nc.sync.dma_start(
                clf[:], cl.ap().rearrange("(c p) k -> p c k", p=128))
            nc.vector.tensor_copy(clh[:], clf[:])
            # clusters2 in d=4p+c layout (matches vlad output partitioning);
            # not needed until the post stage, so SWDGE order is fine.
            nc.gpsimd.dma_start(
                c2n[:], c2.ap().rearrange("(p c) k -> p c k", c=4))

            # ---- phase A: transposes + assignment matmul + BN stats ----
            with tc.tile_pool(name="psA", bufs=2, space="PSUM") as psA:
                # separate banks so the token-sum group can run while the
                # sum-of-squares group is still accumulating (start=True
                # clears a whole bank's has_written bits)
                pstat_q = psA.tile([1, KG], F32, name="pstat_q",
                                   tag="st_q", bufs=1)
                pstat_s = psA.tile([1, KG], F32, name="pstat_s",
                                   tag="st_s", bufs=1)

                NQ = nt // 2            # quarter-groups (2 tiles each)
                xtbufs = {}
                p1bufs = {}

                def xt_dst(q):
                    # quarters 8-23 (tiles 16-47) persist in xTall for the BC
                    # re-matmul; the rest roll through small buffers
                    if 16 <= q < 32:
                        return xTall, slice(8 * (q - 16), 8 * (q - 16) + 8)
                    xt = work.tile([128, 8, 128], F16, name=f"xt{q}",
                                   tag="xt", bufs=LAG + 2)
                    return xt, slice(0, 8)

                def produce(q):
                    # block e = 4j + c holds x[tile 2q+j, 128c:128c+128]^T
                    dst, sl = xt_dst(q)
                    if q in XBAR_QUARTERS:
                        nc.sync.dma_start(dst[:, sl, :],
                                          xh[:, 2 * q:2 * (q + 1), :],
                                          transpose=True)
                    else:
                        pxt = psA.tile([128, 8, 128], F16, name="pxt",
                                       tag="pxt", bufs=PXT_BUFS)
                        for j in range(2):
                            t = 2 * q + j
                            for c in range(4):
                                nc.tensor.transpose(
                                    pxt[:, 4 * j + c, :],
                                    xh[:, t, 128 * c:128 * (c + 1)], idn[:])
                        # batched PSUM->SBUF evacuation; alternate DVE/Act
                        if q % 2 == 0:
                            nc.vector.tensor_copy(dst[:, sl, :], pxt[:])
                        else:
                            nc.scalar.activation(dst[:, sl, :], pxt[:],
                                                 ACTF.Copy)
                    xtbufs[q] = (dst, sl.start)

                def consume(q):
                    src_t, base = xtbufs.pop(q)
                    if q % 2 == 0:
                        p1bufs[q // 2] = psA.tile([128, 4, KG], F32,
                                                  name="p1", tag="p1", bufs=2)
                    p1 = p1bufs[q // 2]
                    for j in range(2):
                        for c in range(4):
                            nc.tensor.matmul(
                                p1[:, 2 * (q % 2) + j, :],
                                src_t[:, base + 4 * j + c, :],
                                clh[:, c, :], start=(c == 0), stop=(c == 3),
                                skip_group_check=True)
                    if q % 2 == 1:
                        h = q // 2
                        sl = slice(4 * h, 4 * (h + 1))
                        # asq is a rolling 4-half window (stats consume it
                        # within the 3-half emission lag)
                        sla = slice(4 * (h % 4), 4 * (h % 4) + 4)
                        nc.scalar.activation(assn[:, sl, :], p1[:], ACTF.Copy)
                        if h == NH - 1:
                            # Act square: DVE is backlogged at phase-A end
                            with nc.allow_low_precision("fp16 stats sq"):
                                nc.scalar.activation(asq[:, sla, :],
                                                     assn[:, sl, :],
                                                     ACTF.Square)
                        else:
                            nc.vector.tensor_tensor(asq[:, sla, :],
                                                    assn[:, sl, :],
                                                    assn[:, sl, :],
                                                    op=OP.mult)

                def stats(h):
                    for j in range(4):
                        t = 4 * h + j
                        nc.tensor.matmul(pstat_q[:], ones16[:],
                                         asq[:, t % 16, :],
                                         start=(t == 0), stop=(t == nt - 1),
                                         skip_group_check=True)
                        if t >= 3 * nt // 4:
                            nc.tensor.matmul(pstat_s[:], ones16[:],
                                             assn[:, t, :],
                                             start=(t == 3 * nt // 4),
                                             stop=False,
                                             skip_group_check=True)

                sacc = persist.tile([128, KG], F16, name="sacc")

                def ssum(c):
                    # DVE free-axis partial sum of assn over 16 tiles
                    with nc.allow_low_precision("fp16 stats partials"):
                        if c == 0:
                            nc.vector.tensor_reduce(
                                sacc[:],
                                assn[:, :16, :].rearrange("p t k -> p k t"),
                                axis=AX.X, op=OP.add)
                            return
                        red = work.tile([128, KG], F16, name="red", tag="red",
                                        bufs=2)
                        nc.vector.tensor_reduce(
                            red[:],
                            assn[:, 16 * c:16 * (c + 1), :]
                            .rearrange("p t k -> p k t"),
                            axis=AX.X, op=OP.add)
                        nc.vector.tensor_tensor(sacc[:], sacc[:], red[:],
                                                op=OP.add)

                # Stats matmuls are emitted 3 half-groups behind the assn
                # evacuations they read: the PE queue is in-order, so a stats
                # matmul whose Act/DVE evacuation hasn't retired yet would
                # stall the whole PE pipeline.
                stats_done = 0
                ssum_done = 0
                for q in range(NQ + LAG):
                    if q < NQ:
                        produce(q)
                    if q >= LAG:
                        cq = q - LAG
                        consume(cq)
                        ready_h = (cq + 1) // 2 - 3
                        while stats_done < ready_h:
                            stats(stats_done)
                            stats_done += 1
                        while ssum_done < min(3, ready_h // 4):
                            ssum(ssum_done)
                            ssum_done += 1
                while stats_done < NH:
                    stats(stats_done)
                    stats_done += 1
                while ssum_done < 3:
                    ssum(ssum_done)
                    ssum_done += 1
                # close the token-sum group with the DVE partial (tiles 0-47)
                nc.tensor.matmul(pstat_s[:], ones16[:], sacc[:],
                                 start=False, stop=True,
                                 skip_group_check=True)

                # ---- neck: stats all-reduce + BN parameters ----
                # stats_sb layout: [sum_sq (q), sum (s)]
                nc.vector.tensor_copy(stats_sb[:, :KG], pstat_q[:])
                nc.vector.tensor_copy(stats_sb[:, KG:], pstat_s[:])

            if with_collective:
                nc.sync.dma_start(stats_in[:], stats_sb[:])
                nc.gpsimd.collective_compute(
                    "AllReduce", OP.add,
                    replica_groups=[list(range(n_cores))],
                    ins=[stats_in.opt()], outs=[stats_out.opt()])
                nc.sync.dma_start(stats_g[:], stats_out[:])
            else:
                # single-core stand-in for the collective hop
                nc.sync.dma_start(stats_g[:], stats_sb[:])

            t_s2 = work.tile([1, KG], F32, name="t_s2", tag="sv2", bufs=4)
            t_vr = work.tile([1, KG], F32, name="t_vr", tag="sv2", bufs=4)
            t_ln = work.tile([1, KG], F32, name="t_ln", tag="sv2", bufs=4)
            t_sc = work.tile([1, KG], F32, name="t_sc", tag="sv2", bufs=4)
            t_mc = work.tile([1, KG], F32, name="t_mc", tag="sv2", bufs=4)
            inv_n = 1.0 / float(total_tok)
            # var = inv_n*(q - inv_n*s^2); rsqrt via exp(-0.5 ln(.)) with the
            # inv_n factor folded into the Ln's scale operand
            q_row, s_row = stats_g[:, :KG], stats_g[:, KG:]
            nc.vector.tensor_tensor(t_s2[:], s_row, s_row, op=OP.mult)
            nc.vector.scalar_tensor_tensor(t_vr[:], t_s2[:], -inv_n, q_row,
                                           op0=OP.mult, op1=OP.add)
            nc.scalar.activation(t_ln[:], t_vr[:], ACTF.Ln, bias=eps_sb[:],
                                 scale=inv_n)
            nc.scalar.activation(t_sc[:], t_ln[:], ACTF.Exp, scale=-0.5)
            with nc.allow_low_precision("fp16 bn scale"):
                nc.vector.tensor_tensor(ss16[:, :KG], t_sc[:], gamma[:],
                                        op=OP.mult)
            # shift = beta - (inv_n*s)*scale_f32*gamma; use fp16 scale copy
            with nc.allow_low_precision("fp16 bn shift"):
                nc.vector.scalar_tensor_tensor(t_mc[:], s_row, inv_n,
                                               ss16[:, :KG],
                                               op0=OP.mult, op1=OP.mult)
                nc.vector.tensor_tensor(ss16[:, KG:], beta[:], t_mc[:],
                                        op=OP.subtract)
            nc.gpsimd.partition_broadcast(bcB[:], ss16[:])
            scale_b = bcB[:, :KG].rearrange("p (a k) -> p a k", a=1)
            shift_b = bcB[:, KG:].rearrange("p (a k) -> p a k", a=1)
            # clusters pre-scaled by the BN scale: z = xT@clp + shift equals
            # scale*(x@cl) + shift, letting PE redo the scale/shift for some
            # batches and unload the DVE-bound softmax phase
            nc.vector.tensor_tensor(
                clp[:], clh[:],
                bcB[:, :KG].rearrange("p (a k) -> p a k", a=1)
                .to_broadcast([128, 4, KG]), op=OP.mult)

            # ---- phase BC: softmax + vlad + normalize, per batch ----
            with (
                tc.tile_pool(name="psB", bufs=2, space="PSUM") as psB,
                tc.tile_pool(name="elem", bufs=2) as elem,
                tc.tile_pool(name="vpost", bufs=2) as vpost,
            ):
                state = {}
                tebufs = {}

                def te_chunk(t0, n, pool_add=False):
                    # te = exp(scale*assn + shift) for token tiles [t0,t0+n)
                    te = elem.tile([128, n, KG], F16, name="te",
                                   tag=f"te{t0}_{n}", bufs=1)
                    if n == TPB and t0 // TPB in (2, 3):
                        # PE path: re-matmul with pre-scaled clusters into a
                        # shift-prefilled PSUM, Exp evacuates straight from
                        # PSUM. Rebalances work from the DVE-bound softmax
                        # onto the PE, which idles during this phase.
                        for g4 in range(n // 4):
                            zt = psB.tile([128, 4, KG], F32, name="zt",
                                          tag="zt", bufs=3)
                            for j in range(4):
                                t = t0 + 4 * g4 + j
                                qq, jj = divmod(t - 2 * TPB, 2)
                                nc.tensor.matmul(
                                    zt[:, j, :], ones_row[:], ss16[:, KG:],
                                    start=True, stop=False,
                                    skip_group_check=True)
                                for c in range(4):
                                    nc.tensor.matmul(
                                        zt[:, j, :],
                                        xTall[:, 8 * qq + 4 * jj + c, :],
                                        clp[:, c, :], start=False,
                                        stop=(c == 3),
                                        skip_group_check=True)
                            nc.scalar.activation(
                                te[:, 4 * g4:4 * (g4 + 1), :], zt[:],
                                ACTF.Exp)
                        tebufs[t0] = te
                        return
                    nc.vector.tensor_tensor(
                        te[:], assn[:, t0:t0 + n, :],
                        scale_b.to_broadcast([128, n, KG]), op=OP.mult)
                    eng = nc.gpsimd if pool_add else nc.vector
                    eng.tensor_tensor(
                        te[:], te[:], shift_b.to_broadcast([128, n, KG]),
                        op=OP.add)
                    nc.scalar.activation(te[:], te[:], ACTF.Exp)
                    tebufs[t0] = te

                def sm_chunk(t0, n):
                    # normalize: sm = te / sum_k te, dropping ghosts
                    te = tebufs.pop(t0)
                    # pairwise-add tree at fp16 2x before the 1x reduce
                    dh = work.tile([128, n, KG // 2], F16, name="dh",
                                   tag=f"dh{n}", bufs=2)
                    dh2 = work.tile([128, n, KG // 4], F16, name="dh2",
                                    tag=f"dh2{n}", bufs=2)
                    with nc.allow_low_precision("fp16 softmax denom"):
                        nc.vector.tensor_tensor(dh[:], te[:, :, :KG // 2],
                                                te[:, :, KG // 2:], op=OP.add)
                        nc.vector.tensor_tensor(dh2[:], dh[:, :, :KG // 4],
                                                dh[:, :, KG // 4:], op=OP.add)
                    denom = work.tile([128, n], F16, name="denom", tag=f"dn{n}",
                                      bufs=2)
                    with nc.allow_low_precision("fp16 softmax denom"):
                        nc.vector.tensor_reduce(denom[:], dh2[:], axis=AX.X,
                                                op=OP.add)
                    recip = work.tile([128, n], F16, name="recip", tag=f"rc{n}",
                                      bufs=2)
                    with nc.allow_low_precision("fp16 softmax recip"):
                        nc.vector.reciprocal(recip[:], denom[:])
                    s0 = t0 % (2 * TPB)
                    nc.vector.tensor_tensor(
                        sm[:, s0:s0 + n, :], te[:, :, :K],
                        recip[:].rearrange("p (t a) -> p t a", a=1)
                        .to_broadcast([128, n, K]), op=OP.mult)

                # chunking: small first chunks so the first vlad matmuls can
                # start early; full batches later for low op overhead
                CHUNKS = [(0, 1), (1, 1), (2, 2), (4, 4), (8, 8)] + [
                    (b * TPB, TPB) for b in range(1, b_loc)]

                def mm_stage(b):
                    t0 = b * TPB
                    pv = psB.tile([128, 4, K], F32, name="pv", tag="pv")
                    pas = psB.tile([1, K], F32, name="pas", tag="pas", bufs=1)
                    # a_sum first: its PSUM lands while the vlad c-groups
                    # stream, so av is ready before the last c-group stops
                    for i in range(TPB):
                        nc.tensor.matmul(pas[:], ones16[:],
                                         sm[:, (t0 + i) % (2 * TPB), :],
                                         start=(i == 0), stop=(i == TPB - 1),
                                         skip_group_check=True)
                    pa16 = work.tile([1, K], F16, name="pa16", tag="pa16",
                                     bufs=2)
                    with nc.allow_low_precision("fp16 a_sum"):
                        nc.scalar.activation(pa16[:], pas[:], ACTF.Copy)
                    av = vpost.tile([128, 4, K], F16, name="av", tag="av")
                    if b == b_loc - 1:
                        # last batch: broadcast via PE + DVE to skip the Pool
                        # round-trips on the tail-critical path
                        pamP = psB.tile([128, K], F32, name="pamP",
                                        tag="pamP", bufs=1)
                        nc.tensor.matmul(pamP[:], ones_row[:], pa16[:],
                                         start=True, stop=True,
                                         skip_group_check=True)
                        nc.vector.tensor_tensor(
                            av[:], c2n[:],
                            pamP[:].rearrange("p (a k) -> p a k", a=1)
                            .to_broadcast([128, 4, K]), op=OP.mult)
                    else:
                        pamB = vpost.tile([128, K], F16, name="pamB",
                                          tag="pam")
                        nc.gpsimd.partition_broadcast(pamB[:], pa16[:])
                        nc.gpsimd.tensor_tensor(
                            av[:], c2n[:],
                            pamB[:].rearrange("p (a k) -> p a k", a=1)
                            .to_broadcast([128, 4, K]), op=OP.mult)
                    # vlad: x stationary with d = 4p + c column layout
                    for c in range(4):
                        for i in range(TPB):
                            t = t0 + i
                            nc.tensor.matmul(
                                pv[:, c, :],
                                xh[:, t, c::4],
                                sm[:, t % (2 * TPB), :],
                                start=(i == 0), stop=(i == TPB - 1),
                                skip_group_check=True)
                    state[b] = (pv, av)

                def post_stage(b):
                    pv, av = state.pop(b)
                    v = vpost.tile([128, 4, K], F16, name="v", tag="v")
                    sq = vpost.tile([128, 4, K], F16, name="sq", tag="sq")
                    pnrm = psB.tile([1, K], F32, name="pnrm", tag="pnrm",
                     bufs=1)
                    # halves over the c dim: v/sq/pnrm for c<2 overlap the
                    # c2/c3 vlad matmuls of this batch
                    for hc in range(2):
                        cs = slice(2 * hc, 2 * hc + 2)
                        with nc.allow_low_precision("fp16 vlad residual"):
                            nc.vector.tensor_tensor(v[:, cs, :], pv[:, cs, :],
                                                    av[:, cs, :],
                                                    op=OP.subtract)
                        with nc.allow_low_precision("fp16 norm squares"):
                            nc.scalar.activation(sq[:, cs, :], v[:, cs, :],
                                                 ACTF.Square)
                        for c in range(2 * hc, 2 * hc + 2):
                            nc.tensor.matmul(pnrm[:], ones16[:], sq[:, c, :],
                                             start=(c == 0), stop=(c == 3),
                                             skip_group_check=True)
                    # y = v * rsqrt(64*nrm2): intra-norm and global L2 norm
                    # folded (flat norm is exactly sqrt(64) post intra-norm)
                    rnl = work.tile([1, K], F32, name="rnl", tag="rnl")
                    nc.scalar.activation(rnl[:], pnrm[:], ACTF.Ln, scale=64.0)
                    rn16 = work.tile([1, K], F16, name="rn16", tag="rn16")
                    with nc.allow_low_precision("fp16 norm scale"):
                        nc.scalar.activation(rn16[:], rnl[:], ACTF.Exp,
                                             scale=-0.5)
                    vf = vpost.tile([128, 4, K], F32, name="vf", tag="vf")
                    yb = y[b, :].rearrange("(p c k) -> p c k", p=128, k=K)
                    if b == b_loc - 1:
                        # last batch: PE broadcast + DVE scale + split y
                        # write to shorten the tail-critical chain
                        prnP = psB.tile([128, K], F32, name="prnP",
                                        tag="pamP", bufs=1)
                        nc.tensor.matmul(prnP[:], ones_row[:], rn16[:],
                                         start=True, stop=True,
                                         skip_group_check=True)
                        prnPv = prnP[:].rearrange("p (a k) -> p a k", a=1)
                        for hc in range(2):
                            cs = slice(2 * hc, 2 * hc + 2)
                            nc.vector.tensor_tensor(
                                vf[:, cs, :], v[:, cs, :],
                                prnPv.to_broadcast([128, 2, K]), op=OP.mult)
                            nc.sync.dma_start(yb[:, cs, :], vf[:, cs, :])
                    else:
                        prnB = vpost.tile([128, K], F16, name="prnB",
                                          tag="prn")
                        nc.gpsimd.partition_broadcast(prnB[:], rn16[:])
                        prnBv = prnB[:].rearrange("p (a k) -> p a k", a=1)
                        nc.gpsimd.tensor_tensor(
                            vf[:], v[:], prnBv.to_broadcast([128, 4, K]),
                            op=OP.mult)
                        nc.sync.dma_start(yb[:, :, :], vf[:])

                # Skew-by-one software pipeline: each sm chunk is emitted one
                # te-chunk later so the DVE stream never waits on an Act exp;
                # vlad (PE) and post stages weave in as batches complete.
                nch = len(CHUNKS)
                done_b = 0
                for i in range(nch + 1):
                    if i < nch:
                        t0, n = CHUNKS[i]
                        te_chunk(t0, n, pool_add=False)
                    if i >= 1:
                        t0, n = CHUNKS[i - 1]
                        sm_chunk(t0, n)
                        if (t0 + n) % TPB == 0:     # batch done_b fully sm'd
                            if done_b >= 1:
                                post_stage(done_b - 1)
                            mm_stage(done_b)
                            done_b += 1
                post_stage(b_loc - 1)
    nc.compile()
    return nc


_CACHE = {}


def _get(b_loc, n_cores, with_collective):
    key = (b_loc, n_cores, with_collective)
    if key not in _CACHE:
        _CACHE[key] = build(b_loc, n_cores, with_collective)
    return _CACHE[key]


def make_in_maps(x, clusters, clusters2, bn_gamma, bn_beta, n_cores=N_CORES):
    B = x.shape[0]
    b_loc = B // n_cores
    shared = {
        "clusters": np.ascontiguousarray(clusters, np.float32),
        "clusters2": np.ascontiguousarray(
            np.asarray(clusters2).reshape(D, K), np.float32),
        "bn_gamma": np.ascontiguousarray(
            np.asarray(bn_gamma).reshape(1, KG), np.float32),
        "bn_beta": np.ascontiguousarray(
            np.asarray(bn_beta).reshape(1, KG), np.float32),
    }
    in_maps = []
    for i in range(n_cores):
        m = dict(shared)
        m["x"] = np.ascontiguousarray(
            np.asarray(x[i * b_loc:(i + 1) * b_loc]).reshape(
                b_loc * N_SEQ, D), np.float32)
        in_maps.append(m)
    return in_maps


def kernel(x, clusters, clusters2, bn_gamma, bn_beta):
    B, N, Dd = x.shape
    assert (N, Dd) == (N_SEQ, D) and B % N_CORES == 0
    b_loc = B // N_CORES
    nc = _get(b_loc, N_CORES, True)
    in_maps = make_in_maps(x, clusters, clusters2, bn_gamma, bn_beta)
    res = run_bass_kernel_spmd(nc, in_maps, core_ids=list(range(N_CORES)))
    out = np.concatenate([res.results[i]["y"] for i in range(N_CORES)], axis=0)
    return out


# revision 73
# speedup vs baseline: 1.0309x; 1.0033x over previous
"""NetVLAD-style vq_codebook kernel for 8 Trainium2 NeuronCores.

Reference computation (per full input):
  assn = BN(x @ clusters); softmax over 80 clusters, drop 16 ghosts
  vlad[b,d,k] = sum_n assn[b,n,k] x[b,n,d] - a_sum[b,k]*clusters2[d,k]
  intra-normalize over d, flatten, global L2 normalize -> (B, D*K)

Sharding: data-parallel over batch B (B/8 batches per core). BatchNorm
statistics (sum and sum-of-squares per cluster column) are all-reduced
across the 8 cores (2*80 floats). Everything else is local.

Key structure (v2, redesigned around the engine cost model):
 - x cast-loaded fp32->fp16 by SWDGE DMA in token-partition layout.
 - x^T (d-partition) via PE transposes (is_transpose matmuls writing
   fp16 PSUM), software-pipelined at 2-tile granularity and batch-
   evacuated to SBUF by DVE and Act, alternating per quarter-group.
 - assignment matmul per token tile: 4 accumulating (128x128)@(128x80)
   fp16 matmuls. BN sum-of-squares via a long PE ones-matmul group;
   BN sums via DVE free-axis reduces + a PE partition reduce, sharing
   one PSUM bank with strictly sequential accumulation groups.
 - softmax: scale/shift as fp16 2x DVE tensor-tensor ops, Exp on Act
   (one activation table for ln/exp/copy/square -> a single load),
   pairwise-halved fp16 denominator, 1/sqrt as exp(-0.5*ln(x)).
 - vlad with x stationary in a d=4p+c column layout so the final DMA
   writes 1KB-contiguous runs; a_sum accumulated directly as [1,64]
   before the vlad groups so a_sum*clusters2 overlaps them.
 - global L2 norm folded analytically: after intra-normalization the
   flat norm is exactly sqrt(64), so y = v * rsqrt(64*nrm2[k]).
 - one serial neck (stats hop + BN math) between the assignment pass
   and the softmax/vlad pass; batch-0 softmax runs in small chunks so
   the first vlad matmuls start early.
"""

import sys

for _p in ("/opt/trn_rl_repo", "/root/.axon_site/_ro/trn_rl_repo"):
    if _p not in sys.path:
        sys.path.insert(0, _p)

import numpy as np

import concourse.bacc as bacc
import concourse.mybir as mybir
import concourse.tile as tile
from concourse.bass_utils import run_bass_kernel_spmd

F32 = mybir.dt.float32
F16 = mybir.dt.float16
AX = mybir.AxisListType
OP = mybir.AluOpType
ACTF = mybir.ActivationFunctionType

N_CORES = 8
D = 512
KG = 80          # clusters + ghosts
K = 64           # real clusters
N_SEQ = 2048
TPB = N_SEQ // 128   # token tiles per batch = 16
BN_EPS = 1e-5

# Tunables
import os as _os
XBAR_QUARTERS = tuple(
    int(v) for v in _os.environ.get("K_XBAR", "30,31").split(",") if v)
LAG = int(_os.environ.get("K_LAG", "5"))
LOADS = tuple(int(v) for v in _os.environ.get("K_LOADS", "4,4").split(","))
PXT_BUFS = int(_os.environ.get("K_PXT", "4"))


def build(b_loc=4, n_cores=N_CORES, with_collective=True):
    """Build the per-core program. b_loc = batches per core."""
    nt = b_loc * TPB                # token tiles per core
    tok = nt * 128                  # tokens per core
    total_tok = tok * n_cores       # global token count for BN stats
    NH = nt // 4                    # half-groups (4 tiles each)

    nc = bacc.Bacc("TRN2", target_bir_lowering=False, debug=False,
                   dynamic_dma_scratch_size=65536)

    x = nc.declare_dram_parameter("x", [tok, D], F32, isOutput=False)
    cl = nc.declare_dram_parameter("clusters", [D, KG], F32, isOutput=False)
    c2 = nc.declare_dram_parameter("clusters2", [D, K], F32, isOutput=False)
    gam = nc.declare_dram_parameter("bn_gamma", [1, KG], F32, isOutput=False)
    bet = nc.declare_dram_parameter("bn_beta", [1, KG], F32, isOutput=False)
    y = nc.declare_dram_parameter("y", [b_loc, D * K], F32, isOutput=True)

    eye_c = nc.inline_tensor(np.eye(128, dtype=np.float16), name="c_eye")

    with tile.TileContext(nc) as tc:
        with (
            tc.tile_pool(name="persist", bufs=1) as persist,
            tc.tile_pool(name="work", bufs=4) as work,
            tc.tile_pool(name="dram", bufs=1, space="DRAM") as dram,
        ):
            # ---- persistent SBUF tensors ----
            xh = persist.tile([128, nt, D], F16, name="xh")
            # persistent x^T only for tiles 32-63 (quarters 16-31): the BC
            # re-matmul for batches 2-3 re-reads exactly these
            xTall = persist.tile([128, 2 * nt, 128], F16,
                                 name="xTall")
            clp = persist.tile([128, 4, KG], F16, name="clp")
            assn = persist.tile([128, nt, KG], F16, name="assn")
            asq = persist.tile([128, 16, KG], F16, name="asq")
            # rolling two-batch window: vlad(b) trails sm(b) by <1 batch
            sm = persist.tile([128, 2 * TPB, K], F16, name="sm")
            idn = persist.tile([128, 128], F16, name="idn")
            clh = persist.tile([128, 4, KG], F16, name="clh")
            c2n = persist.tile([128, 4, K], F16, name="c2n")
            ones16 = persist.tile([128, 1], F16, name="ones16")
            ones_row = persist.tile([1, 128], F16, name="ones_row")
            gamma = persist.tile([1, KG], F32, name="gamma")
            beta = persist.tile([1, KG], F32, name="beta")
            ss16 = persist.tile([1, 2 * KG], F16, name="ss16")
            bcB = persist.tile([128, 2 * KG], F16, name="bcB")
            stats_sb = persist.tile([1, 2 * KG], F32, name="stats_sb")
            stats_g = persist.tile([1, 2 * KG], F32, name="stats_g")
            actwarm = persist.tile([1, 1], F32, name="actwarm")
            eps_sb = persist.tile([1, 1], F32, name="eps_sb")

            stats_in = dram.tile([1, 2 * KG], F32, name="stats_in")
            stats_out = dram.tile([1, 2 * KG], F32, name="stats_out")

            # ---- phase 0: constants + x load/cast ----
            nc.sync.dma_start(gamma[:], gam[:, :])
            nc.sync.dma_start(beta[:], bet[:, :])
            nc.sync.dma_start(idn[:], eye_c.ap()[:, :])
            nc.vector.memset(ones16[:], 1.0)
            nc.vector.memset(ones_row[:], 1.0)
            nc.vector.memset(eps_sb[:], BN_EPS)
            # Pre-load the one activation table covering every function this
            # kernel uses (ln/exp/copy/square), so the table-load inserter
            # doesn't alternate between ln-only and exp-only sets. Best
            # effort: fall back to automatic insertion if the set is absent.
            try:
                from concourse.hw_specs import get_activation_tables
                tabs = get_activation_tables(nc.m.arch)
                set_id = list(tabs).index("natural_log_exp_and_others")
                nc.scalar.add_instruction(mybir.InstLoadActFuncSet(
                    name=nc.get_next_instruction_name(),
                    engine=mybir.EngineType.Activation,
                    act_func_set_id=set_id, ins=[], outs=[]))
            except (ImportError, ValueError, KeyError):
                pass
            # Touch the activation engine early so any residual table load
            # happens off the critical path.
            nc.scalar.activation(actwarm[:], gamma[:, :1], ACTF.Ln)

            # x cast-DMA (SWDGE casts fp32->fp16 in the DMA engines; HBM
            # read is the real cost). Small first chunks start the PE
            # transpose pipeline sooner.
            xr = x.ap().rearrange("(t p) d -> p t d", p=128)
            t0 = 0
            for sz in LOADS + (8,) * ((nt - sum(LOADS)) // 8):
                nc.gpsimd.dma_start(
                    xh[:, t0:t0 + sz, :], xr[:, t0:t0 + sz, :])
                t0 += sz
            assert t0 == nt
            # clusters via HWDGE (fp32) + DVE cast: the Pool/SWDGE queue is
            # saturated by the x loads, and clh is needed early.
            clf = work.tile([128, 4, KG], F32, name="clf", tag="clf", bufs=1)
            nc.sync.dma_start(
                clf[:], cl.ap().rearrange("(c p) k -> p c k", p=128))
            nc.vector.tensor_copy(clh[:], clf[:])
            # clusters2 in d=4p+c layout (matches vlad output partitioning);
            # not needed until the post stage, so SWDGE order is fine.
            nc.gpsimd.dma_start(
                c2n[:], c2.ap().rearrange("(p c) k -> p c k", c=4))

            # ---- phase A: transposes + assignment matmul + BN stats ----
            with tc.tile_pool(name="psA", bufs=2, space="PSUM") as psA:
                # separate banks so the token-sum group can run while the
                # sum-of-squares group is still accumulating (start=True
                # clears a whole bank's has_written bits)
                pstat_q = psA.tile([1, KG], F32, name="pstat_q",
                                   tag="st_q", bufs=1)
                pstat_s = psA.tile([1, KG], F32, name="pstat_s",
                                   tag="st_s", bufs=1)

                NQ = nt // 2            # quarter-groups (2 tiles each)
                xtbufs = {}
                p1bufs = {}

                def xt_dst(q):
                    # quarters 8-23 (tiles 16-47) persist in xTall for the BC
                    # re-matmul; the rest roll through small buffers
                    if 16 <= q < 32:
                        return xTall, slice(8 * (q - 16), 8 * (q - 16) + 8)
                    xt = work.tile([128, 8, 128], F16, name=f"xt{q}",
                                   tag="xt", bufs=LAG + 2)
                    return xt, slice(0, 8)

                def produce(q):
                    # block e = 4j + c holds x[tile 2q+j, 128c:128c+128]^T
                    dst, sl = xt_dst(q)
                    if q in XBAR_QUARTERS:
                        nc.sync.dma_start(dst[:, sl, :],
                                          xh[:, 2 * q:2 * (q + 1), :],
                                          transpose=True)
                    else:
                        pxt = psA.tile([128, 8, 128], F16, name="pxt",
                                       tag="pxt", bufs=PXT_BUFS)
                        for j in range(2):
                            t = 2 * q + j
                            for c in range(4):
                                nc.tensor.transpose(
                                    pxt[:, 4 * j + c, :],
                                    xh[:, t, 128 * c:128 * (c + 1)], idn[:])
                        # batched PSUM->SBUF evacuation; alternate DVE/Act
                        if q % 2 == 0:
                            nc.vector.tensor_copy(dst[:, sl, :], pxt[:])
                        else:
                            nc.scalar.activation(dst[:, sl, :], pxt[:],
                                                 ACTF.Copy)
                    xtbufs[q] = (dst, sl.start)

                def consume(q):
                    src_t, base = xtbufs.pop(q)
                    if q % 2 == 0:
                        p1bufs[q // 2] = psA.tile([128, 4, KG], F32,
                                                  name="p1", tag="p1", bufs=2)
                    p1 = p1bufs[q // 2]
                    for j in range(2):
                        for c in range(4):
                            nc.tensor.matmul(
                                p1[:, 2 * (q % 2) + j, :],
                                src_t[:, base + 4 * j + c, :],
                                clh[:, c, :], start=(c == 0), stop=(c == 3),
                                skip_group_check=True)
                    if q % 2 == 1:
                        h = q // 2
                        sl = slice(4 * h, 4 * (h + 1))
                        # asq is a rolling 4-half window (stats consume it
                        # within the 3-half emission lag)
                        sla = slice(4 * (h % 4), 4 * (h % 4) + 4)
                        nc.scalar.activation(assn[:, sl, :], p1[:], ACTF.Copy)
                        if h == NH - 1:
                            # Act square: DVE is backlogged at phase-A end
                            with nc.allow_low_precision("fp16 stats sq"):
                                nc.scalar.activation(asq[:, sla, :],
                                                     assn[:, sl, :],
                                                     ACTF.Square)
                        else:
                            nc.vector.tensor_tensor(asq[:, sla, :],
                                                    assn[:, sl, :],
                                                    assn[:, sl, :],
                                                    op=OP.mult)

                def stats(h):
                    for j in range(4):
                        t = 4 * h + j
                        nc.tensor.matmul(pstat_q[:], ones16[:],
                                         asq[:, t % 16, :],
                                         start=(t == 0), stop=(t == nt - 1),
                                         skip_group_check=True)
                        if t >= 3 * nt // 4:
                            nc.tensor.matmul(pstat_s[:], ones16[:],
                                             assn[:, t, :],
                                             start=(t == 3 * nt // 4),
                                             stop=False,
                                             skip_group_check=True)

                sacc = persist.tile([128, KG], F16, name="sacc")

                def ssum(c):
                    # DVE free-axis partial sum of assn over 16 tiles
                    with nc.allow_low_precision("fp16 stats partials"):
                        if c == 0:
                            nc.vector.tensor_reduce(
                                sacc[:],
                                assn[:, :16, :].rearrange("p t k -> p k t"),
                                axis=AX.X, op=OP.add)
                            return
                        red = work.tile([128, KG], F16, name="red", tag="red",
                                        bufs=2)
                        nc.vector.tensor_reduce(
                            red[:],
                            assn[:, 16 * c:16 * (c + 1), :]
                            .rearrange("p t k -> p k t"),
                            axis=AX.X, op=OP.add)
                        nc.vector.tensor_tensor(sacc[:], sacc[:], red[:],
                                                op=OP.add)

                # Stats matmuls are emitted 3 half-groups behind the assn
                # evacuations they read: the PE queue is in-order, so a stats
                # matmul whose Act/DVE evacuation hasn't retired yet would
                # stall the whole PE pipeline.
                stats_done = 0
                ssum_done = 0
                for q in range(NQ + LAG):
                    if q < NQ:
                        produce(q)
                    if q >= LAG:
                        cq = q - LAG
                        consume(cq)
                        ready_h = (cq + 1) // 2 - 3
                        while stats_done < ready_h:
                            stats(stats_done)
                            stats_done += 1
                        while ssum_done < min(3, ready_h // 4):
                            ssum(ssum_done)
                            ssum_done += 1
                while stats_done < NH:
                    stats(stats_done)
                    stats_done += 1
                while ssum_done < 3:
                    ssum(ssum_done)
                    ssum_done += 1
                # close the token-sum group with the DVE partial (tiles 0-47)
                nc.tensor.matmul(pstat_s[:], ones16[:], sacc[:],
                                 start=False, stop=True,
                                 skip_group_check=True)

                # ---- neck: stats all-reduce + BN parameters ----
                # stats_sb layout: [sum_sq (q), sum (s)]
                nc.vector.tensor_copy(stats_sb[:, :KG], pstat_q[:])
                nc.vector.tensor_copy(stats_sb[:, KG:], pstat_s[:])

            if with_collective:
                nc.sync.dma_start(stats_in[:], stats_sb[:])
                nc.gpsimd.collective_compute(
                    "AllReduce", OP.add,
                    replica_groups=[list(range(n_cores))],
                    ins=[stats_in.opt()], outs=[stats_out.opt()])
                nc.sync.dma_start(stats_g[:], stats_out[:])
            else:
                # single-core stand-in for the collective hop
                nc.sync.dma_start(stats_g[:], stats_sb[:])

            t_s2 = work.tile([1, KG], F32, name="t_s2", tag="sv2", bufs=4)
            t_vr = work.tile([1, KG], F32, name="t_vr", tag="sv2", bufs=4)
            t_ln = work.tile([1, KG], F32, name="t_ln", tag="sv2", bufs=4)
            t_sc = work.tile([1, KG], F32, name="t_sc", tag="sv2", bufs=4)
            t_mc = work.tile([1, KG], F32, name="t_mc", tag="sv2", bufs=4)
            inv_n = 1.0 / float(total_tok)
            # var = inv_n*(q - inv_n*s^2); rsqrt via exp(-0.5 ln(.)) with the
            # inv_n factor folded into the Ln's scale operand
            q_row, s_row = stats_g[:, :KG], stats_g[:, KG:]
            nc.vector.tensor_tensor(t_s2[:], s_row, s_row, op=OP.mult)
            nc.vector.scalar_tensor_tensor(t_vr[:], t_s2[:], -inv_n, q_row,
                                           op0=OP.mult, op1=OP.add)
            nc.scalar.activation(t_ln[:], t_vr[:], ACTF.Ln, bias=eps_sb[:],
                                 scale=inv_n)
            nc.scalar.activation(t_sc[:], t_ln[:], ACTF.Exp, scale=-0.5)
            with nc.allow_low_precision("fp16 bn scale"):
                nc.vector.tensor_tensor(ss16[:, :KG], t_sc[:], gamma[:],
                                        op=OP.mult)
            # shift = beta - (inv_n*s)*scale_f32*gamma; use fp16 scale copy
            with nc.allow_low_precision("fp16 bn shift"):
                nc.vector.scalar_tensor_tensor(t_mc[:], s_row, inv_n,
                                               ss16[:, :KG],
                                               op0=OP.mult, op1=OP.mult)
                nc.vector.tensor_tensor(ss16[:, KG:], beta[:], t_mc[:],
                                        op=OP.subtract)
            nc.gpsimd.partition_broadcast(bcB[:], ss16[:])
            scale_b = bcB[:, :KG].rearrange("p (a k) -> p a k", a=1)
            shift_b = bcB[:, KG:].rearrange("p (a k) -> p a k", a=1)
            # clusters pre-scaled by the BN scale: z = xT@clp + shift equals
            # scale*(x@cl) + shift, letting PE redo the scale/shift for some
            # batches and unload the DVE-bound softmax phase
            nc.vector.tensor_tensor(
                clp[:], clh[:],
                bcB[:, :KG].rearrange("p (a k) -> p a k", a=1)
                .to_broadcast([128, 4, KG]), op=OP.mult)

            # ---- phase BC: softmax + vlad + normalize, per batch ----
            with (
                tc.tile_pool(name="psB", bufs=2, space="PSUM") as psB,
                tc.tile_pool(name="elem", bufs=2) as elem,
                tc.tile_pool(name="vpost", bufs=2) as vpost,
            ):
                state = {}
                tebufs = {}

                def te_chunk(t0, n, pool_add=False):
                    # te = exp(scale*assn + shift) for token tiles [t0,t0+n)
                    te = elem.tile([128, n, KG], F16, name="te",
                                   tag=f"te{t0}_{n}", bufs=1)
                    if n == TPB and t0 // TPB in (2, 3):
                        # PE path: re-matmul with pre-scaled clusters into a
                        # shift-prefilled PSUM, Exp evacuates straight from
                        # PSUM. Rebalances work from the DVE-bound softmax
                        # onto the PE, which idles during this phase.
                        for g4 in range(n // 4):
                            zt = psB.tile([128, 4, KG], F32, name="zt",
                                          tag="zt", bufs=3)
                            for j in range(4):
                                t = t0 + 4 * g4 + j
                                qq, jj = divmod(t - 2 * TPB, 2)
                                nc.tensor.matmul(
                                    zt[:, j, :], ones_row[:], ss16[:, KG:],
                                    start=True, stop=False,
                                    skip_group_check=True)
                                for c in range(4):
                                    nc.tensor.matmul(
                                        zt[:, j, :],
                                        xTall[:, 8 * qq + 4 * jj + c, :],
                                        clp[:, c, :], start=False,
                                        stop=(c == 3),
                                        skip_group_check=True)
                            nc.scalar.activation(
                                te[:, 4 * g4:4 * (g4 + 1), :], zt[:],
                                ACTF.Exp)
                        tebufs[t0] = te
                        return
                    nc.vector.tensor_tensor(
                        te[:], assn[:, t0:t0 + n, :],
                        scale_b.to_broadcast([128, n, KG]), op=OP.mult)
                    eng = nc.gpsimd if pool_add else nc.vector
                    eng.tensor_tensor(
                        te[:], te[:], shift_b.to_broadcast([128, n, KG]),
                        op=OP.add)
                    nc.scalar.activation(te[:], te[:], ACTF.Exp)
                    tebufs[t0] = te

                def sm_chunk(t0, n):
                    # normalize: sm = te / sum_k te, dropping ghosts
                    te = tebufs.pop(t0)
                    # pairwise-add tree at fp16 2x before the 1x reduce
                    dh = work.tile([128, n, KG // 2], F16, name="dh",
                                   tag=f"dh{n}", bufs=2)
                    dh2 = work.tile([128, n, KG // 4], F16, name="dh2",
                                    tag=f"dh2{n}", bufs=2)
                    with nc.allow_low_precision("fp16 softmax denom"):
                        nc.vector.tensor_tensor(dh[:], te[:, :, :KG // 2],
                                                te[:, :, KG // 2:], op=OP.add)
                        nc.vector.tensor_tensor(dh2[:], dh[:, :, :KG // 4],
                                                dh[:, :, KG // 4:], op=OP.add)
                    denom = work.tile([128, n], F16, name="denom", tag=f"dn{n}",
                                      bufs=2)
                    with nc.allow_low_precision("fp16 softmax denom"):
                        nc.vector.tensor_reduce(denom[:], dh2[:], axis=AX.X,
                                                op=OP.add)
                    recip = work.tile([128, n], F16, name="recip", tag=f"rc{n}",
                                      bufs=2)
                    with nc.allow_low_precision("fp16 softmax recip"):
                        nc.vector.reciprocal(recip[:], denom[:])
                    s0 = t0 % (2 * TPB)
                    nc.vector.tensor_tensor(
                        sm[:, s0:s0 + n, :], te[:, :, :K],
                        recip[:].rearrange("p (t a) -> p t a", a=1)
                        .to_broadcast([128, n, K]), op=OP.mult)

                # chunking: small first chunks so the first vlad matmuls can
                # start early; full batches later for low op overhead
                CHUNKS = [(0, 1), (1, 1), (2, 2), (4, 4), (8, 8)] + [
                    (b * TPB, TPB) for b in range(1, b_loc)]

                def mm_stage(b):
                    t0 = b * TPB
                    pv = psB.tile([128, 4, K], F32, name="pv", tag="pv")
                    pas = psB.tile([1, K], F32, name="pas", tag="pas", bufs=1)
                    # a_sum first: its PSUM lands while the vlad c-groups
                    # stream, so av is ready before the last c-group stops
                    for i in range(TPB):
                        nc.tensor.matmul(pas[:], ones16[:],
                                         sm[:, (t0 + i) % (2 * TPB), :],
                                         start=(i == 0), stop=(i == TPB - 1),
                                         skip_group_check=True)
                    pa16 = work.tile([1, K], F16, name="pa16", tag="pa16",
                                     bufs=2)
                    with nc.allow_low_precision("fp16 a_sum"):
                        nc.scalar.activation(pa16[:], pas[:], ACTF.Copy)
                    av = vpost.tile([128, 4, K], F16, name="av", tag="av")
                    if b == b_loc - 1:
                        # last batch: broadcast via PE + DVE to skip the Pool
                        # round-trips on the tail-critical path
                        pamP = psB.tile([128, K], F32, name="pamP",
                                        tag="pamP", bufs=1)
                        nc.tensor.matmul(pamP[:], ones_row[:], pa16[:],
                                         start=True, stop=True,
                                         skip_group_check=True)
                        nc.vector.tensor_tensor(
                            av[:], c2n[:],
                            pamP[:].rearrange("p (a k) -> p a k", a=1)
                            .to_broadcast([128, 4, K]), op=OP.mult)
                    else:
                        pamB = vpost.tile([128, K], F16, name="pamB",
                                          tag="pam")
                        nc.gpsimd.partition_broadcast(pamB[:], pa16[:])
                        nc.gpsimd.tensor_tensor(
                            av[:], c2n[:],
                            pamB[:].rearrange("p (a k) -> p a k", a=1)
                            .to_broadcast([128, 4, K]), op=OP.mult)
                    # vlad: x stationary with d = 4p + c column layout
                    for c in range(4):
                        for i in range(TPB):
                            t = t0 + i
                            nc.tensor.matmul(
                                pv[:, c, :],
                                xh[:, t, c::4],
                                sm[:, t % (2 * TPB), :],
                                start=(i == 0), stop=(i == TPB - 1),
                                skip_group_check=True)
                    state[b] = (pv, av)

                def post_stage(b):
                    pv, av = state.pop(b)
                    v = vpost.tile([128, 4, K], F16, name="v", tag="v")
                    sq = vpost.tile([128, 4, K], F16, name="sq", tag="sq")
                    pnrm = psB.tile([1, K], F32, name="pnrm", tag="pnrm",
                     bufs=1)
                    # halves over the c dim: v/sq/pnrm for c<2 overlap the
                    # c2/c3 vlad matmuls of this batch
                    for hc in range(2):
                        cs = slice(2 * hc, 2 * hc + 2)
                        with nc.allow_low_precision("fp16 vlad residual"):
                            nc.vector.tensor_tensor(v[:, cs, :], pv[:, cs, :],
                                                    av[:, cs, :],
                                                    op=OP.subtract)
                        with nc.allow_low_precision("fp16 norm squares"):
                            nc.scalar.activation(sq[:, cs, :], v[:, cs, :],
                                                 ACTF.Square)
                        for c in range(2 * hc, 2 * hc + 2):
                            nc.tensor.matmul(pnrm[:], ones16[:], sq[:, c, :],
                                             start=(c == 0), stop=(c == 3),
                                             skip_group_check=True)
                    # y = v * rsqrt(64*nrm2): intra-norm and global L2 norm
                    # folded (flat norm is exactly sqrt(64) post intra-norm)
                    rnl = work.tile([1, K], F32, name="rnl", tag="rnl")
                    nc.scalar.activation(rnl[:], pnrm[:], ACTF.Ln, scale=64.0)
                    rn16 = work.tile([1, K], F16, name="rn16", tag="rn16")
                    with nc.allow_low_precision("fp16 norm scale"):
                        nc.scalar.activation(rn16[:], rnl[:], ACTF.Exp,
                                             scale=-0.5)
                    vf = vpost.tile([128, 4, K], F32, name="vf", tag="vf")
                    yb = y[b, :].rearrange("(p c k) -> p c k", p=128, k=K)
                    if b == b_loc - 1:
                        # last batch: PE broadcast + DVE scale + split y
                        # write to shorten the tail-critical chain
                        prnP = psB.tile([128, K], F32, name="prnP",
                                        tag="pamP", bufs=1)
                        nc.tensor.matmul(prnP[:], ones_row[:], rn16[:],
                                         start=True, stop=True,
                                         skip_group_check=True)
                        prnPv = prnP[:].rearrange("p (a k) -> p a k", a=1)
                        for hc in range(2):
                            cs = slice(2 * hc, 2 * hc + 2)
                            nc.vector.tensor_tensor(
                                vf[:, cs, :], v[:, cs, :],
                                prnPv.to_broadcast([128, 2, K]), op=OP.mult)
                            nc.sync.dma_start(yb[:, cs, :], vf[:, cs, :])
                    else:
                        prnB = vpost.tile([128, K], F16, name="prnB",
                                          tag="prn")
                        nc.gpsimd.partition_broadcast(prnB[:], rn16[:])
                        prnBv = prnB[:].rearrange("p (a k) -> p a k", a=1)
                        nc.gpsimd.tensor_tensor(
                            vf[:], v[:], prnBv.to_broadcast([128, 4, K]),
                            op=OP.mult)
                        nc.sync.dma_start(yb[:, :, :], vf[:])

                # Skew-by-one software pipeline: each sm chunk is emitted one
                # te-chunk later so the DVE stream never waits on an Act exp;
                # vlad (PE) and post stages weave in as batches complete.
                nch = len(CHUNKS)
                done_b = 0
                for i in range(nch + 1):
                    if i < nch:
                        t0, n = CHUNKS[i]
                        te_chunk(t0, n, pool_add=False)
                    if i >= 1:
                        t0, n = CHUNKS[i - 1]
                        sm_chunk(t0, n)
                        if (t0 + n) % TPB == 0:     # batch done_b fully sm'd
                            if done_b >= 1:
                                post_stage(done_b - 1)
                            mm_stage(done_b)
                            done_b += 1
                post_stage(b_loc - 1)
    nc.compile()
    return nc


_CACHE = {}


def _get(b_loc, n_cores, with_collective):
    key = (b_loc, n_cores, with_collective)
    if key not in _CACHE:
        _CACHE[key] = build(b_loc, n_cores, with_collective)
    return _CACHE[key]


def make_in_maps(x, clusters, clusters2, bn_gamma, bn_beta, n_cores=N_CORES):
    B = x.shape[0]
    b_loc = B // n_cores
    shared = {
        "clusters": np.ascontiguousarray(clusters, np.float32),
        "clusters2": np.ascontiguousarray(
            np.asarray(clusters2).reshape(D, K), np.float32),
        "bn_gamma": np.ascontiguousarray(
            np.asarray(bn_gamma).reshape(1, KG), np.float32),
        "bn_beta": np.ascontiguousarray(
            np.asarray(bn_beta).reshape(1, KG), np.float32),
    }
    in_maps = []
    for i in range(n_cores):
        m = dict(shared)
        m["x"] = np.ascontiguousarray(
            np.asarray(x[i * b_loc:(i + 1) * b_loc]).reshape(
                b_loc * N_SEQ, D), np.float32)
        in_maps.append(m)
    return in_maps


def kernel(x, clusters, clusters2, bn_gamma, bn_beta):
    B, N, Dd = x.shape
    assert (N, Dd) == (N_SEQ, D) and B % N_CORES == 0
    b_loc = B // N_CORES
    nc = _get(b_loc, N_CORES, True)
    in_maps = make_in_maps(x, clusters, clusters2, bn_gamma, bn_beta)
    res = run_bass_kernel_spmd(nc, in_maps, core_ids=list(range(N_CORES)))
    out = np.concatenate([res.results[i]["y"] for i in range(N_CORES)], axis=0)
    return out
